# revision 27
# baseline (speedup 1.0000x reference)
"""Trainium2 Bass kernel for nn_CrossAttention (B=4, Nq=Nk=2048, D=1024, H=16).

Sharding: 8 cores = (batch b in 0..3) x (head-group hg in 0..1), 8 heads/core.
Each core gets its batch's query/context plus the column slice of Wq/Wk/Wv and
row slice of Wo for its 8 heads; LayerNorm params are replicated.  Host sums
the two head-group partial outputs per batch and adds bo.

Per-core pipeline (all matmuls bf16 with fp32 PSUM accumulation):
  LN (fp32, bn_stats) -> PE transpose (bf16) -> Q/K/V projections
  (Q,K in [dout, tok] layout; V in [tok, dout] with a ones column appended)
  -> S^T = K Q^T tiles (2 heads packed in the 128-row PE array)
  -> exp on ScalarE with the 1/sqrt(dh) scale folded in
  -> AV matmul with M=65 (row 64 = softmax denominator Z, for free)
  -> normalize via GPSIMD partition_broadcast of 1/Z + DVE multiply
  -> Wo row-slice matmul -> fp32 partial output.
"""

import numpy as np

import concourse.bass as bass
import concourse.mybir as mybir
import concourse.tile as tile
from concourse import bacc
from concourse.masks import make_identity

P = 128
N_TOK = 2048          # tokens per batch (both Nq and Nk)
D = 1024              # model dim
KS = D // P           # 8 contraction subtiles
DG = 512              # per-core projection width (8 heads * 64)
NM = DG // P          # 4 output blocks / head-pair groups
NH = 8                # heads per core
HD = 64
NT = N_TOK // P       # 16 token tiles
NCH = N_TOK // 512    # 4 token chunks of 512
SCALE = HD ** -0.5
EPS = 1e-5

F32 = mybir.dt.float32
BF16 = mybir.dt.bfloat16
_UNIQ = [0]
_EXP_ON_DVE = [False]  # timing-diagnostic only: wrong numerics
_PHASE_ONLY = [None]   # None | "proj" | "dma"  (timing-diagnostic only)


def _build_program(ln_affine=True, repeat=1, hw_loop=0):
    nc = bacc.Bacc("TRN2", target_bir_lowering=False, debug=False)

    q_in = nc.dram_tensor("q_in", (N_TOK, D), F32, kind="ExternalInput")
    c_in = nc.dram_tensor("c_in", (N_TOK, D), F32, kind="ExternalInput")
    wq = nc.dram_tensor("wq", (D, DG), F32, kind="ExternalInput")
    wk = nc.dram_tensor("wk", (D, DG), F32, kind="ExternalInput")
    wv = nc.dram_tensor("wv", (D, DG), F32, kind="ExternalInput")
    wo = nc.dram_tensor("wo", (DG, D), F32, kind="ExternalInput")
    bq_d = nc.dram_tensor("bq", (DG,), F32, kind="ExternalInput")
    bk_d = nc.dram_tensor("bk", (DG,), F32, kind="ExternalInput")
    bv_d = nc.dram_tensor("bv", (DG,), F32, kind="ExternalInput")
    gq_d = nc.dram_tensor("gq", (D,), F32, kind="ExternalInput")
    btq_d = nc.dram_tensor("btq", (D,), F32, kind="ExternalInput")
    gkv_d = nc.dram_tensor("gkv", (D,), F32, kind="ExternalInput")
    btkv_d = nc.dram_tensor("btkv", (D,), F32, kind="ExternalInput")
    y_out = nc.dram_tensor("y_out", (N_TOK, D), F32, kind="ExternalOutput")

    import contextlib

    with tile.TileContext(nc) as tc:
        loop_ctx = tc.For_i(0, hw_loop, 1) if hw_loop else None
        with (loop_ctx if loop_ctx is not None else contextlib.nullcontext()):
         for _rep in range(repeat):
            _UNIQ[0] += 1
            with (
                tc.tile_pool(name="persist", bufs=1) as persist,
                tc.tile_pool(name="wopool", bufs=1) as wopool,
                tc.tile_pool(name="ps_proj", bufs=2, space="PSUM") as ps_proj,
            ):
                # ---------------- persistent tensors ----------------
                qt = [persist.tile([P, NM, 512], BF16, tag=f"qt{c}",
                                   name=f"qt{c}_{_UNIQ[0]}")
                      for c in range(NCH)]   # Q^T per token chunk
                kt = persist.tile([P, NM, N_TOK], BF16, tag="kt")   # K^T
                vs = persist.tile([P, NT, NH, HD + 1], BF16, tag="vs")
                os_t = [persist.tile([P, NM, 512], BF16, tag=f"os{c}",
                                     name=f"os{c}_{_UNIQ[0]}")
                        for c in range(NCH)]
                nc.vector.memset(vs[:, :, :, HD:HD + 1], 1.0)
                wo_bf = wopool.tile([P, NM, D], BF16, tag="wo_bf")

                if _PHASE_ONLY[0] == "dma":
                    _dma_only(nc, tc, q_in, c_in, wq, wk, wv, wo, y_out)
                    continue
                _proj_phase(nc, tc, q_in, c_in, wq, wk, wv, wo,
                            bq_d, bk_d, bv_d, gq_d, btq_d, gkv_d, btkv_d,
                            qt, kt, vs, wo_bf, ps_proj, ln_affine)
                if _PHASE_ONLY[0] == "proj":
                    yt0 = persist.tile([P, 512], F32, tag="y0",
                                       name=f"y0_{_UNIQ[0]}")
                    nc.vector.tensor_copy(out=yt0[:, :],
                                          in_=kt[:, 0, 0:512])
                    nc.sync.dma_start(out=y_out[0:P, 0:512], in_=yt0)
                    continue
                _attn_phase(nc, tc, qt, kt, vs, os_t, wo_bf, ps_proj, y_out)

    nc.finalize()
    return nc


def _proj_phase(nc, tc, q_in, c_in, wq, wk, wv, wo,
                bq_d, bk_d, bv_d, gq_d, btq_d, gkv_d, btkv_d,
                qt, kt, vs, wo_bf, ps_proj, ln_affine):
    with (
        tc.tile_pool(name="consts", bufs=1) as consts,
        tc.tile_pool(name="weights", bufs=1) as wpool,
        tc.tile_pool(name="wstage", bufs=4) as wstage,
        tc.tile_pool(name="xload", bufs=6) as xpool,
        tc.tile_pool(name="stats", bufs=4) as stats,
        tc.tile_pool(name="lntmp", bufs=2) as lntmp,
        tc.tile_pool(name="lnout", bufs=5) as lnpool,
        tc.tile_pool(name="lnT", bufs=3) as lntpool,
        tc.tile_pool(name="ps_tr", bufs=2, space="PSUM") as ps_tr,
    ):
            # ---------------- constants ----------------
            ident = consts.tile([P, P], BF16, tag="ident")
            make_identity(nc, ident)
            eps_t = consts.tile([P, 1], F32, tag="eps")
            nc.vector.memset(eps_t, EPS)
            if ln_affine:
                gq_b = consts.tile([P, D], F32, tag="gq_b")
                nc.gpsimd.dma_start(out=gq_b,
                                    in_=gq_d[None, :].to_broadcast((P, D)))
                btq_b = consts.tile([P, D], F32, tag="btq_b")
                nc.gpsimd.dma_start(out=btq_b,
                                    in_=btq_d[None, :].to_broadcast((P, D)))
                gkv_b = consts.tile([P, D], F32, tag="gkv_b")
                nc.gpsimd.dma_start(out=gkv_b,
                                    in_=gkv_d[None, :].to_broadcast((P, D)))
                btkv_b = consts.tile([P, D], F32, tag="btkv_b")
                nc.gpsimd.dma_start(out=btkv_b,
                                    in_=btkv_d[None, :].to_broadcast((P, D)))
            else:
                gq_b = btq_b = gkv_b = btkv_b = None
            bv_b = consts.tile([P, DG], F32, tag="bv_b")
            nc.gpsimd.dma_start(out=bv_b, in_=bv_d[None, :].to_broadcast((P, DG)))
            bq_c = consts.tile([P, NM], F32, tag="bq_c")
            nc.sync.dma_start(out=bq_c, in_=bq_d.rearrange("(m p) -> p m", p=P))
            bk_c = consts.tile([P, NM], F32, tag="bk_c")
            nc.sync.dma_start(out=bk_c, in_=bk_d.rearrange("(m p) -> p m", p=P))

            # ---------------- weights (fp32 -> bf16) ----------------
            wq_bf = wpool.tile([P, KS, DG], BF16, tag="wq_bf")
            wk_bf = wpool.tile([P, KS, DG], BF16, tag="wk_bf")
            wv_bf = wpool.tile([P, KS, DG], BF16, tag="wv_bf")
            for w_d, w_bf in ((wq, wq_bf), (wk, wk_bf), (wv, wv_bf)):
                w_r = w_d.rearrange("(s p) n -> p s n", p=P)
                for s in range(KS):
                    st = wstage.tile([P, D], F32, tag="wst")
                    (nc.sync if s % 2 == 0 else nc.scalar).dma_start(
                        out=st[:, :DG], in_=w_r[:, s, :])
                    nc.vector.tensor_copy(out=w_bf[:, s, :], in_=st[:, :DG])
            wo_r = wo.rearrange("(m p) n -> p m n", p=P)
            for m in range(NM):
                st = wstage.tile([P, D], F32, tag="wst")
                nc.sync.dma_start(out=st, in_=wo_r[:, m, :])
                nc.vector.tensor_copy(out=wo_bf[:, m, :], in_=st)

            # ---------------- LN + transpose + projections ----------------
            def ln_tile(src, t, g_b, b_b):
                """LayerNorm token tile t of src -> [128, 1024] bf16 tile."""
                x = xpool.tile([P, D], F32, tag="x")
                dma_eng = nc.sync if t % 2 == 0 else nc.scalar
                dma_eng.dma_start(out=x, in_=src[t * P:(t + 1) * P, :])
                st = stats.tile([P, 2, 6], F32, tag="bnst")
                nc.vector.bn_stats(out=st[:, 0, :], in_=x[:, 0:512])
                nc.vector.bn_stats(out=st[:, 1, :], in_=x[:, 512:1024])
                mv = stats.tile([P, 2], F32, tag="mv")
                nc.vector.bn_aggr(out=mv, in_=st)
                lnv = stats.tile([P, 1], F32, tag="lnv")
                nc.scalar.activation(out=lnv, in_=mv[:, 1:2],
                                     func=mybir.ActivationFunctionType.Ln,
                                     bias=eps_t)
                rstd = stats.tile([P, 1], F32, tag="rstd")
                nc.scalar.activation(out=rstd, in_=lnv,
                                     func=mybir.ActivationFunctionType.Exp,
                                     scale=-0.5)
                lnt = lnpool.tile([P, D], BF16, tag="ln")
                if not ln_affine:
                    nc.vector.tensor_scalar(out=lnt, in0=x, scalar1=mv[:, 0:1],
                                            scalar2=rstd,
                                            op0=mybir.AluOpType.subtract,
                                            op1=mybir.AluOpType.mult)
                    return lnt
                xc = lntmp.tile([P, D], F32, tag="xc")
                nc.vector.tensor_scalar(out=xc, in0=x, scalar1=mv[:, 0:1],
                                        scalar2=rstd,
                                        op0=mybir.AluOpType.subtract,
                                        op1=mybir.AluOpType.mult)
                xg = lntmp.tile([P, D], F32, tag="xg")
                nc.vector.tensor_tensor(out=xg, in0=xc, in1=g_b,
                                        op=mybir.AluOpType.mult)
                nc.vector.tensor_tensor(out=lnt, in0=xg, in1=b_b,
                                        op=mybir.AluOpType.add)
                return lnt

            def transpose_chunk(ln_tiles):
                """4 LN tiles ([128 tok, 1024 feat]) -> lnT [128 feat, 8, 512 tok]."""
                lnT = lntpool.tile([P, KS, 512], BF16, tag="lnT")
                for s in range(KS):
                    pt = ps_tr.tile([P, 512], BF16, tag="tr")
                    for tl in range(4):
                        nc.tensor.transpose(pt[:, tl * P:(tl + 1) * P],
                                            ln_tiles[tl][:, s * P:(s + 1) * P],
                                            ident)
                    nc.vector.tensor_copy(out=lnT[:, s, :], in_=pt)
                return lnT

            # context: K^T, V
            for c in range(NCH):
                ln_tiles = [ln_tile(c_in, 4 * c + tl, gkv_b, btkv_b)
                            for tl in range(4)]
                lnT = transpose_chunk(ln_tiles)
                for m in range(NM):
                    pp = ps_proj.tile([P, 512], F32, tag="pp")
                    for s in range(KS):
                        nc.tensor.matmul(pp, lhsT=wk_bf[:, s, m * P:(m + 1) * P],
                                         rhs=lnT[:, s, :],
                                         start=(s == 0), stop=(s == KS - 1))
                    nc.vector.tensor_scalar_add(
                        out=kt[:, m, c * 512:(c + 1) * 512], in0=pp,
                        scalar1=bk_c[:, m:m + 1])
                for tl in range(4):
                    t = 4 * c + tl
                    pp = ps_proj.tile([P, 512], F32, tag="pp")
                    for s in range(KS):
                        nc.tensor.matmul(pp, lhsT=lnT[:, s, tl * P:(tl + 1) * P],
                                         rhs=wv_bf[:, s, :],
                                         start=(s == 0), stop=(s == KS - 1))
                    nc.vector.tensor_tensor(
                        out=vs[:, t, :, 0:HD],
                        in0=pp.rearrange("p (h d) -> p h d", h=NH),
                        in1=bv_b.rearrange("p (h d) -> p h d", h=NH),
                        op=mybir.AluOpType.add)

            # query: Q^T
            for c in range(NCH):
                ln_tiles = [ln_tile(q_in, 4 * c + tl, gq_b, btq_b)
                            for tl in range(4)]
                lnT = transpose_chunk(ln_tiles)
                for m in range(NM):
                    pp = ps_proj.tile([P, 512], F32, tag="pp")
                    for s in range(KS):
                        nc.tensor.matmul(pp, lhsT=wq_bf[:, s, m * P:(m + 1) * P],
                                         rhs=lnT[:, s, :],
                                         start=(s == 0), stop=(s == KS - 1))
                    nc.vector.tensor_scalar_add(
                        out=qt[c][:, m, :], in0=pp,
                        scalar1=bq_c[:, m:m + 1])


def _attn_phase(nc, tc, qt, kt, vs, os_t, wo_bf, ps_proj, y_out):
    with (
        tc.tile_pool(name="exp", bufs=2) as exppool,
        tc.tile_pool(name="smalls", bufs=2) as smalls,
        tc.tile_pool(name="yout", bufs=3) as ypool,
        tc.tile_pool(name="ps_s", bufs=2, space="PSUM") as ps_s,
        tc.tile_pool(name="ps_av", bufs=2, space="PSUM") as ps_av,
    ):
        uq = _UNIQ[0]

        def emit_av_chunk(prev, kg):
            c0, j0, exp_pair, avs = prev
            for hl in range(2):
                for k2 in range(2):
                    ki = kg * 2 + k2
                    nc.tensor.matmul(avs[hl], lhsT=vs[:, ki, 2 * j0 + hl, :],
                                     rhs=exp_pair[hl][:, ki, :],
                                     start=(ki == 0), stop=(ki == NT - 1),
                                     skip_group_check=True)

        def emit_normalize(prev):
            c0, j0, exp_pair, avs = prev
            for hl in range(2):
                av = avs[hl]
                zrow = smalls.tile([1, 512], F32, tag="zrow",
                                   name=f"zrow{c0}_{j0}_{hl}_{uq}")
                nc.vector.reciprocal(out=zrow, in_=av[HD:HD + 1, :])
                rinv = smalls.tile([HD, 512], F32, tag="rinv",
                                   name=f"rinv{c0}_{j0}_{hl}_{uq}")
                nc.gpsimd.partition_broadcast(rinv, zrow)
                nc.vector.tensor_tensor(
                    out=os_t[c0][hl * HD:(hl + 1) * HD, j0, :],
                    in0=av[0:HD, :], in1=rinv,
                    op=mybir.AluOpType.mult)

        def emit_wo_group(c0, g):
            tl, dc = g // 2, g % 2
            t = 4 * c0 + tl
            pp = ps_proj.tile([P, 512], F32, tag="pp",
                              name=f"wopp{c0}_{g}_{uq}")
            for m in range(NM):
                nc.tensor.matmul(
                    pp, lhsT=os_t[c0][:, m, tl * P:(tl + 1) * P],
                    rhs=wo_bf[:, m, dc * 512:(dc + 1) * 512],
                    start=(m == 0), stop=(m == NM - 1),
                    skip_group_check=True)
            yt = ypool.tile([P, 512], F32, tag="y",
                            name=f"yt{c0}_{g}_{uq}")
            nc.vector.tensor_copy(out=yt, in_=pp)
            nc.sync.dma_start(
                out=y_out[t * P:(t + 1) * P, dc * 512:(dc + 1) * 512],
                in_=yt)

        prev = None
        wo_pending = []   # (c, next_group_idx)
        for c in range(NCH):
            for j in range(NM):
                exp_pair = [exppool.tile([P, NT, 512], BF16, tag=f"exp{hl}",
                                         name=f"exp{hl}_{c}_{j}_{uq}")
                            for hl in range(2)]
                for kg in range(8):
                    ps_pair = [ps_s.tile([P, 2, 512], F32, tag="psS",
                                         name=f"psS{hl}_{c}_{j}_{kg}_{uq}")
                               for hl in range(2)]
                    for k2 in range(2):
                        ki = kg * 2 + k2
                        for hl in range(2):
                            rows = slice(hl * HD, (hl + 1) * HD)
                            nc.tensor.matmul(
                                ps_pair[hl][:, k2, :],
                                lhsT=kt[rows, j, ki * P:(ki + 1) * P],
                                rhs=qt[c][rows, j, :],
                                start=True, stop=True,
                                skip_group_check=True)
                    for hl in range(2):
                        if _EXP_ON_DVE[0]:
                            nc.vector.tensor_copy(
                                out=exp_pair[hl][:, kg * 2:kg * 2 + 2, :],
                                in_=ps_pair[hl][:, :, :])
                        else:
                            nc.scalar.activation(
                                out=exp_pair[hl][:, kg * 2:kg * 2 + 2, :],
                                in_=ps_pair[hl][:, :, :],
                                func=mybir.ActivationFunctionType.Exp,
                                scale=SCALE)
                    if prev is not None:
                        emit_av_chunk(prev, kg)
                    if wo_pending and kg in (3, 7):
                        c0, g = wo_pending[0]
                        emit_wo_group(c0, g)
                        if g + 1 >= 8:
                            wo_pending.pop(0)
                        else:
                            wo_pending[0] = (c0, g + 1)
                if prev is not None:
                    emit_normalize(prev)
                    if prev[1] == NM - 1:      # finished batch-chunk prev[0]
                        wo_pending.append((prev[0], 0))
                avs = [ps_av.tile([HD + 1, 512], F32, tag="av",
                                  name=f"av{c}_{j}_{hl}_{uq}")
                       for hl in range(2)]
                prev = (c, j, exp_pair, avs)
        # drain: AV + normalize of the last (c,j), then remaining Wo groups
        for kg in range(8):
            emit_av_chunk(prev, kg)
        emit_normalize(prev)
        wo_pending.append((prev[0], 0))
        for c0, g0 in list(wo_pending):
            for g in range(g0, 8):
                emit_wo_group(c0, g)


_CACHE = {}


def _get_exec(ln_affine=True, repeat=1, hw_loop=0):
    """Build the Bass program once and wrap it in a reusable jitted executor."""
    key = ("exec", ln_affine, repeat, hw_loop)
    if key in _CACHE:
        return _CACHE[key]

    import jax
    from jax.sharding import Mesh, PartitionSpec
    from jax.experimental.shard_map import shard_map
    from concourse import bass2jax

    nc = _build_program(ln_affine=ln_affine, repeat=repeat, hw_loop=hw_loop)
    bass2jax.install_neuronx_cc_hook()

    partition_name = (nc.partition_id_tensor.name
                      if nc.partition_id_tensor else None)
    in_names, out_names, out_avals, zero_shapes = [], [], [], []
    for alloc in nc.m.functions[0].allocations:
        if not isinstance(alloc, mybir.MemoryLocationSet):
            continue
        name = alloc.memorylocations[0].name
        if alloc.kind == "ExternalInput":
            if name != partition_name:
                in_names.append(name)
        elif alloc.kind == "ExternalOutput":
            shape = tuple(alloc.tensor_shape)
            dtype = mybir.dt.np(alloc.dtype)
            out_names.append(name)
            out_avals.append(jax.core.ShapedArray(shape, dtype))
            zero_shapes.append((shape, dtype))
    n_params = len(in_names)
    n_outs = len(out_avals)
    all_names = list(in_names) + list(out_names)
    if partition_name is not None:
        all_names.append(partition_name)
    donate = tuple(range(n_params, n_params + n_outs))

    def _body(*args):
        operands = list(args)
        if partition_name is not None:
            operands.append(bass2jax.partition_id_tensor())
        outs = bass2jax._bass_exec_p.bind(
            *operands,
            out_avals=tuple(out_avals),
            in_names=tuple(all_names),
            out_names=tuple(out_names),
            lowering_input_output_aliases=(),
            sim_require_finite=True,
            sim_require_nnan=True,
            nc=nc,
        )
        return tuple(outs)

    n_cores = 8
    devices = jax.devices()[:n_cores]
    mesh = Mesh(np.asarray(devices), ("core",))
    in_specs = (PartitionSpec("core"),) * (n_params + n_outs)
    out_specs = (PartitionSpec("core"),) * n_outs
    sharded = jax.jit(
        shard_map(_body, mesh=mesh, in_specs=in_specs, out_specs=out_specs,
                  check_rep=False),
        donate_argnums=donate, keep_unused=True)

    def execute(in_maps):
        per_core = [[np.ascontiguousarray(np.asarray(m[name], np.float32))
                     for name in in_names] for m in in_maps]
        concat_in = [np.concatenate([per_core[cc][i] for cc in range(n_cores)],
                                    axis=0) for i in range(n_params)]
        concat_zeros = [np.zeros((n_cores * s[0], *s[1:]), d)
                        for (s, d) in zero_shapes]
        out_arrs = sharded(*concat_in, *concat_zeros)
        return [
            {name: np.asarray(out_arrs[i]).reshape(n_cores, *out_avals[i].shape)[cc]
             for i, name in enumerate(out_names)}
            for cc in range(n_cores)
        ]

    _CACHE[key] = execute
    _CACHE[("parts", ln_affine, repeat, hw_loop)] = {
        "sharded": sharded, "in_names": in_names, "n_params": n_params,
        "out_names": out_names, "out_avals": out_avals,
        "zero_shapes": zero_shapes, "mesh": mesh, "n_cores": n_cores,
        "body": _body, "in_specs": in_specs, "out_specs": out_specs,
        "donate": donate,
    }
    return execute


def _time_exec(in_maps, iters=5, ln_affine=True, repeat=1, hw_loop=0):
    """Time the sharded executable with device-resident inputs (seconds)."""
    import time
    import jax
    from jax.sharding import NamedSharding, PartitionSpec

    _get_exec(ln_affine=ln_affine, repeat=repeat, hw_loop=hw_loop)
    parts = _CACHE[("parts", ln_affine, repeat, hw_loop)]
    sharded = parts["sharded"]
    n_cores = parts["n_cores"]
    sh = NamedSharding(parts["mesh"], PartitionSpec("core"))
    per_core = [[np.ascontiguousarray(np.asarray(m[name], np.float32))
                 for name in parts["in_names"]] for m in in_maps]
    concat_in = [np.concatenate([per_core[cc][i] for cc in range(n_cores)],
                                axis=0) for i in range(parts["n_params"])]
    in_dev = [jax.device_put(a, sh) for a in concat_in]
    jax.block_until_ready(in_dev)
    times = []
    for _ in range(iters):
        z_dev = [jax.device_put(
                     np.zeros((n_cores * s[0], *s[1:]), d), sh)
                 for (s, d) in parts["zero_shapes"]]
        jax.block_until_ready(z_dev)
        t0 = time.perf_counter()
        out = sharded(*in_dev, *z_dev)
        jax.block_until_ready(out)
        times.append(time.perf_counter() - t0)
        del out
    return times


def _ln_is_identity(inputs):
    return all(
        np.all(np.asarray(inputs[k], np.float32) == v)
        for k, v in (("gq", 1.0), ("betq", 0.0), ("gkv", 1.0), ("betkv", 0.0))
    )


def _make_in_maps(inputs):
    q = np.asarray(inputs["query"], np.float32)
    c = np.asarray(inputs["context"], np.float32)
    Wq = np.asarray(inputs["Wq"], np.float32)
    Wk = np.asarray(inputs["Wk"], np.float32)
    Wv = np.asarray(inputs["Wv"], np.float32)
    Wo = np.asarray(inputs["Wo"], np.float32)
    bq = np.asarray(inputs["bq"], np.float32)
    bk = np.asarray(inputs["bk"], np.float32)
    bv = np.asarray(inputs["bv"], np.float32)
    gq = np.asarray(inputs["gq"], np.float32)
    btq = np.asarray(inputs["betq"], np.float32)
    gkv = np.asarray(inputs["gkv"], np.float32)
    btkv = np.asarray(inputs["betkv"], np.float32)
    in_maps = []
    for core in range(8):
        b, hg = core // 2, core % 2
        sl = slice(hg * DG, (hg + 1) * DG)
        in_maps.append({
            "q_in": q[b], "c_in": c[b],
            "wq": Wq[:, sl], "wk": Wk[:, sl], "wv": Wv[:, sl],
            "wo": Wo[sl, :],
            "bq": bq[sl], "bk": bk[sl], "bv": bv[sl],
            "gq": gq, "btq": btq, "gkv": gkv, "btkv": btkv,
        })
    return in_maps


def kernel(**inputs):
    execute = _get_exec(ln_affine=not _ln_is_identity(inputs))
    in_maps = _make_in_maps(inputs)
    results = execute(in_maps)
    bo = np.asarray(inputs["bo"], np.float32)
    B = 4
    out = np.empty((B, N_TOK, D), np.float32)
    for b in range(B):
        out[b] = results[2 * b]["y_out"] + results[2 * b + 1]["y_out"] + bo
    return out


def _dma_only(nc, tc, q_in, c_in, wq, wk, wv, wo, y_out):
    with tc.tile_pool(name="dml", bufs=4) as pool:
        acc = pool.tile([P, D], F32, tag="acc")
        for src in (q_in, c_in):
            for t in range(NT):
                x = pool.tile([P, D], F32, tag="dx")
                nc.sync.dma_start(out=x, in_=src[t * P:(t + 1) * P, :])
                nc.vector.tensor_copy(out=acc, in_=x)
        for w_d in (wq, wk, wv):
            w_r = w_d.rearrange("(s p) n -> p s n", p=P)
            for s_ in range(KS):
                x = pool.tile([P, D], F32, tag="dx")
                nc.sync.dma_start(out=x[:, :DG], in_=w_r[:, s_, :])
                nc.vector.tensor_copy(out=acc, in_=x)
        wo_r = wo.rearrange("(m p) n -> p m n", p=P)
        for m in range(NM):
            x = pool.tile([P, D], F32, tag="dx")
            nc.sync.dma_start(out=x, in_=wo_r[:, m, :])
            nc.vector.tensor_copy(out=acc, in_=x)
        nc.sync.dma_start(out=y_out[0:P, 0:D], in_=acc)


# revision 30
# speedup vs baseline: 1.0163x; 1.0163x over previous
"""Trainium2 Bass kernel for nn_CrossAttention (B=4, Nq=Nk=2048, D=1024, H=16).

Sharding: 8 cores = (batch b in 0..3) x (head-group hg in 0..1), 8 heads/core.
Each core gets its batch's query/context plus the column slice of Wq/Wk/Wv and
row slice of Wo for its 8 heads; LayerNorm params are replicated.  Host sums
the two head-group partial outputs per batch and adds bo.

Per-core pipeline (all matmuls bf16 with fp32 PSUM accumulation):
  LN (fp32, bn_stats) -> PE transpose (bf16) -> Q/K/V projections
  (Q,K in [dout, tok] layout; V in [tok, dout] with a ones column appended)
  -> S^T = K Q^T tiles (2 heads packed in the 128-row PE array)
  -> exp on ScalarE with the 1/sqrt(dh) scale folded in
  -> AV matmul with M=65 (row 64 = softmax denominator Z, for free)
  -> normalize via GPSIMD partition_broadcast of 1/Z + DVE multiply
  -> Wo row-slice matmul -> fp32 partial output.
"""

import numpy as np

import concourse.bass as bass
import concourse.mybir as mybir
import concourse.tile as tile
from concourse import bacc
from concourse.masks import make_identity

P = 128
N_TOK = 2048          # tokens per batch (both Nq and Nk)
D = 1024              # model dim
KS = D // P           # 8 contraction subtiles
DG = 512              # per-core projection width (8 heads * 64)
NM = DG // P          # 4 output blocks / head-pair groups
NH = 8                # heads per core
HD = 64
NT = N_TOK // P       # 16 token tiles
NCH = N_TOK // 512    # 4 token chunks of 512
SCALE = HD ** -0.5
EPS = 1e-5

F32 = mybir.dt.float32
BF16 = mybir.dt.bfloat16
_UNIQ = [0]
_EXP_ON_DVE = [False]  # timing-diagnostic only: wrong numerics
_PHASE_ONLY = [None]   # None | "proj" | "dma"  (timing-diagnostic only)
_DMA_MODE = [0]


def _build_program(ln_affine=True, repeat=1, hw_loop=0):
    nc = bacc.Bacc("TRN2", target_bir_lowering=False, debug=False)

    q_in = nc.dram_tensor("q_in", (N_TOK, D), F32, kind="ExternalInput")
    c_in = nc.dram_tensor("c_in", (N_TOK, D), F32, kind="ExternalInput")
    wq = nc.dram_tensor("wq", (D, DG), F32, kind="ExternalInput")
    wk = nc.dram_tensor("wk", (D, DG), F32, kind="ExternalInput")
    wv = nc.dram_tensor("wv", (D, DG), F32, kind="ExternalInput")
    wo = nc.dram_tensor("wo", (DG, D), F32, kind="ExternalInput")
    bq_d = nc.dram_tensor("bq", (DG,), F32, kind="ExternalInput")
    bk_d = nc.dram_tensor("bk", (DG,), F32, kind="ExternalInput")
    bv_d = nc.dram_tensor("bv", (DG,), F32, kind="ExternalInput")
    gq_d = nc.dram_tensor("gq", (D,), F32, kind="ExternalInput")
    btq_d = nc.dram_tensor("btq", (D,), F32, kind="ExternalInput")
    gkv_d = nc.dram_tensor("gkv", (D,), F32, kind="ExternalInput")
    btkv_d = nc.dram_tensor("btkv", (D,), F32, kind="ExternalInput")
    y_out = nc.dram_tensor("y_out", (N_TOK, D), F32, kind="ExternalOutput")

    import contextlib

    with tile.TileContext(nc) as tc:
        loop_ctx = tc.For_i(0, hw_loop, 1) if hw_loop else None
        with (loop_ctx if loop_ctx is not None else contextlib.nullcontext()):
         for _rep in range(repeat):
            _UNIQ[0] += 1
            with (
                tc.tile_pool(name="persist", bufs=1) as persist,
                tc.tile_pool(name="wopool", bufs=1) as wopool,
                tc.tile_pool(name="ps_proj", bufs=2, space="PSUM") as ps_proj,
            ):
                # ---------------- persistent tensors ----------------
                qt = [persist.tile([P, NM, 512], BF16, tag=f"qt{c}",
                                   name=f"qt{c}_{_UNIQ[0]}")
                      for c in range(NCH)]   # Q^T per token chunk
                kt = persist.tile([P, NM, N_TOK], BF16, tag="kt")   # K^T
                vs = persist.tile([P, NT, NH, HD + 1], BF16, tag="vs")
                os_t = [persist.tile([P, NM, 512], BF16, tag=f"os{c}",
                                     name=f"os{c}_{_UNIQ[0]}")
                        for c in range(NCH)]
                nc.vector.memset(vs[:, :, :, HD:HD + 1], 1.0)
                wo_bf = wopool.tile([P, NM, D], BF16, tag="wo_bf")

                if _PHASE_ONLY[0] == "dma":
                    _dma_only(nc, tc, q_in, c_in, wq, wk, wv, wo, y_out)
                    continue
                _proj_phase(nc, tc, q_in, c_in, wq, wk, wv, wo,
                            bq_d, bk_d, bv_d, gq_d, btq_d, gkv_d, btkv_d,
                            qt, kt, vs, wo_bf, ps_proj, ln_affine)
                if _PHASE_ONLY[0] == "proj":
                    yt0 = persist.tile([P, 512], F32, tag="y0",
                                       name=f"y0_{_UNIQ[0]}")
                    nc.vector.tensor_copy(out=yt0[:, :],
                                          in_=kt[:, 0, 0:512])
                    nc.sync.dma_start(out=y_out[0:P, 0:512], in_=yt0)
                    continue
                _attn_phase(nc, tc, qt, kt, vs, os_t, wo_bf, ps_proj, y_out)

    nc.finalize()
    return nc


def _proj_phase(nc, tc, q_in, c_in, wq, wk, wv, wo,
                bq_d, bk_d, bv_d, gq_d, btq_d, gkv_d, btkv_d,
                qt, kt, vs, wo_bf, ps_proj, ln_affine):
    with (
        tc.tile_pool(name="consts", bufs=1) as consts,
        tc.tile_pool(name="weights", bufs=1) as wpool,
        tc.tile_pool(name="wstage", bufs=2) as wstage,
        tc.tile_pool(name="xload", bufs=2) as xpool,
        tc.tile_pool(name="stats", bufs=4) as stats,
        tc.tile_pool(name="lntmp", bufs=2) as lntmp,
        tc.tile_pool(name="lnout", bufs=5) as lnpool,
        tc.tile_pool(name="lnT", bufs=2) as lntpool,
        tc.tile_pool(name="ps_tr", bufs=2, space="PSUM") as ps_tr,
    ):
            # ---------------- constants ----------------
            ident = consts.tile([P, P], BF16, tag="ident")
            make_identity(nc, ident)
            eps_t = consts.tile([P, 1], F32, tag="eps")
            nc.vector.memset(eps_t, EPS)
            if ln_affine:
                gq_b = consts.tile([P, D], F32, tag="gq_b")
                nc.gpsimd.dma_start(out=gq_b,
                                    in_=gq_d[None, :].to_broadcast((P, D)))
                btq_b = consts.tile([P, D], F32, tag="btq_b")
                nc.gpsimd.dma_start(out=btq_b,
                                    in_=btq_d[None, :].to_broadcast((P, D)))
                gkv_b = consts.tile([P, D], F32, tag="gkv_b")
                nc.gpsimd.dma_start(out=gkv_b,
                                    in_=gkv_d[None, :].to_broadcast((P, D)))
                btkv_b = consts.tile([P, D], F32, tag="btkv_b")
                nc.gpsimd.dma_start(out=btkv_b,
                                    in_=btkv_d[None, :].to_broadcast((P, D)))
            else:
                gq_b = btq_b = gkv_b = btkv_b = None
            bv_b = consts.tile([P, DG], F32, tag="bv_b")
            nc.gpsimd.dma_start(out=bv_b, in_=bv_d[None, :].to_broadcast((P, DG)))
            bq_c = consts.tile([P, NM], F32, tag="bq_c")
            nc.sync.dma_start(out=bq_c, in_=bq_d.rearrange("(m p) -> p m", p=P))
            bk_c = consts.tile([P, NM], F32, tag="bk_c")
            nc.sync.dma_start(out=bk_c, in_=bk_d.rearrange("(m p) -> p m", p=P))

            # ---------------- weights (fp32 -> bf16) ----------------
            wq_bf = wpool.tile([P, KS, DG], BF16, tag="wq_bf")
            wk_bf = wpool.tile([P, KS, DG], BF16, tag="wk_bf")
            wv_bf = wpool.tile([P, KS, DG], BF16, tag="wv_bf")
            for wi, (w_d, w_bf) in enumerate(
                    ((wq, wq_bf), (wk, wk_bf), (wv, wv_bf))):
                w_r = w_d.rearrange("(a s p) n -> p a s n", p=P, s=4)
                for a in range(2):
                    st = wstage.tile([P, 4, DG], F32, tag="wst")
                    (nc.sync if (wi + a) % 2 == 0 else nc.scalar).dma_start(
                        out=st, in_=w_r[:, a])
                    nc.vector.tensor_copy(
                        out=w_bf[:, a * 4:(a + 1) * 4, :], in_=st)
            wo_r = wo.rearrange("(m p) n -> p m n", p=P)
            st = wstage.tile([P, 4, DG], F32, tag="wst")
            st2 = wstage.tile([P, 4, DG], F32, tag="wst")
            nc.sync.dma_start(out=st, in_=wo_r[:, :, 0:DG])
            nc.scalar.dma_start(out=st2, in_=wo_r[:, :, DG:D])
            nc.vector.tensor_copy(out=wo_bf[:, :, 0:DG], in_=st)
            nc.vector.tensor_copy(out=wo_bf[:, :, DG:D], in_=st2)

            # ---------------- LN + transpose + projections ----------------
            def ln_tile(xb, tl, g_b, b_b):
                """LayerNorm token tile xb[:, tl, :] -> [128, 1024] bf16."""
                x = xb[:, tl, :]
                st = stats.tile([P, 2, 6], F32, tag="bnst")
                nc.vector.bn_stats(out=st[:, 0, :], in_=x[:, 0:512])
                nc.vector.bn_stats(out=st[:, 1, :], in_=x[:, 512:1024])
                mv = stats.tile([P, 2], F32, tag="mv")
                nc.vector.bn_aggr(out=mv, in_=st)
                lnv = stats.tile([P, 1], F32, tag="lnv")
                nc.scalar.activation(out=lnv, in_=mv[:, 1:2],
                                     func=mybir.ActivationFunctionType.Ln,
                                     bias=eps_t)
                rstd = stats.tile([P, 1], F32, tag="rstd")
                nc.scalar.activation(out=rstd, in_=lnv,
                                     func=mybir.ActivationFunctionType.Exp,
                                     scale=-0.5)
                lnt = lnpool.tile([P, D], BF16, tag="ln")
                if not ln_affine:
                    nc.vector.tensor_scalar(out=lnt, in0=x, scalar1=mv[:, 0:1],
                                            scalar2=rstd,
                                            op0=mybir.AluOpType.subtract,
                                            op1=mybir.AluOpType.mult)
                    return lnt
                xc = lntmp.tile([P, D], F32, tag="xc")
                nc.vector.tensor_scalar(out=xc, in0=x, scalar1=mv[:, 0:1],
                                        scalar2=rstd,
                                        op0=mybir.AluOpType.subtract,
                                        op1=mybir.AluOpType.mult)
                xg = lntmp.tile([P, D], F32, tag="xg")
                nc.vector.tensor_tensor(out=xg, in0=xc, in1=g_b,
                                        op=mybir.AluOpType.mult)
                nc.vector.tensor_tensor(out=lnt, in0=xg, in1=b_b,
                                        op=mybir.AluOpType.add)
                return lnt

            def transpose_chunk(ln_tiles):
                """4 LN tiles ([128 tok, 1024 feat]) -> lnT [128 feat, 8, 512 tok]."""
                lnT = lntpool.tile([P, KS, 512], BF16, tag="lnT")
                for s in range(KS):
                    pt = ps_tr.tile([P, 512], BF16, tag="tr")
                    for tl in range(4):
                        nc.tensor.transpose(pt[:, tl * P:(tl + 1) * P],
                                            ln_tiles[tl][:, s * P:(s + 1) * P],
                                            ident)
                    nc.vector.tensor_copy(out=lnT[:, s, :], in_=pt)
                return lnT

            # context: K^T, V
            c_r = c_in.rearrange("(n i p) d -> n p i d", p=P, i=4)
            q_r = q_in.rearrange("(n i p) d -> n p i d", p=P, i=4)
            for c in range(NCH):
                xb = xpool.tile([P, 4, D], F32, tag="xb")
                (nc.sync if c % 2 == 0 else nc.scalar).dma_start(
                    out=xb, in_=c_r[c])
                ln_tiles = [ln_tile(xb, tl, gkv_b, btkv_b)
                            for tl in range(4)]
                lnT = transpose_chunk(ln_tiles)
                for m in range(NM):
                    pp = ps_proj.tile([P, 512], F32, tag="pp")
                    for s in range(KS):
                        nc.tensor.matmul(pp, lhsT=wk_bf[:, s, m * P:(m + 1) * P],
                                         rhs=lnT[:, s, :],
                                         start=(s == 0), stop=(s == KS - 1))
                    nc.vector.tensor_scalar_add(
                        out=kt[:, m, c * 512:(c + 1) * 512], in0=pp,
                        scalar1=bk_c[:, m:m + 1])
                for tl in range(4):
                    t = 4 * c + tl
                    pp = ps_proj.tile([P, 512], F32, tag="pp")
                    for s in range(KS):
                        nc.tensor.matmul(pp, lhsT=lnT[:, s, tl * P:(tl + 1) * P],
                                         rhs=wv_bf[:, s, :],
                                         start=(s == 0), stop=(s == KS - 1))
                    nc.vector.tensor_tensor(
                        out=vs[:, t, :, 0:HD],
                        in0=pp.rearrange("p (h d) -> p h d", h=NH),
                        in1=bv_b.rearrange("p (h d) -> p h d", h=NH),
                        op=mybir.AluOpType.add)

            # query: Q^T
            for c in range(NCH):
                xb = xpool.tile([P, 4, D], F32, tag="xb")
                (nc.sync if c % 2 == 0 else nc.scalar).dma_start(
                    out=xb, in_=q_r[c])
                ln_tiles = [ln_tile(xb, tl, gq_b, btq_b)
                            for tl in range(4)]
                lnT = transpose_chunk(ln_tiles)
                for m in range(NM):
                    pp = ps_proj.tile([P, 512], F32, tag="pp")
                    for s in range(KS):
                        nc.tensor.matmul(pp, lhsT=wq_bf[:, s, m * P:(m + 1) * P],
                                         rhs=lnT[:, s, :],
                                         start=(s == 0), stop=(s == KS - 1))
                    nc.vector.tensor_scalar_add(
                        out=qt[c][:, m, :], in0=pp,
                        scalar1=bq_c[:, m:m + 1])


def _attn_phase(nc, tc, qt, kt, vs, os_t, wo_bf, ps_proj, y_out):
    with (
        tc.tile_pool(name="exp", bufs=2) as exppool,
        tc.tile_pool(name="smalls", bufs=2) as smalls,
        tc.tile_pool(name="yout", bufs=3) as ypool,
        tc.tile_pool(name="ps_s", bufs=2, space="PSUM") as ps_s,
        tc.tile_pool(name="ps_av", bufs=2, space="PSUM") as ps_av,
    ):
        uq = _UNIQ[0]

        def emit_av_chunk(prev, kg):
            c0, j0, exp_pair, avs = prev
            for hl in range(2):
                for k2 in range(2):
                    ki = kg * 2 + k2
                    nc.tensor.matmul(avs[hl], lhsT=vs[:, ki, 2 * j0 + hl, :],
                                     rhs=exp_pair[hl][:, ki, :],
                                     start=(ki == 0), stop=(ki == NT - 1),
                                     skip_group_check=True)

        def emit_normalize(prev):
            c0, j0, exp_pair, avs = prev
            for hl in range(2):
                av = avs[hl]
                zrow = smalls.tile([1, 512], F32, tag="zrow",
                                   name=f"zrow{c0}_{j0}_{hl}_{uq}")
                nc.vector.reciprocal(out=zrow, in_=av[HD:HD + 1, :])
                rinv = smalls.tile([HD, 512], F32, tag="rinv",
                                   name=f"rinv{c0}_{j0}_{hl}_{uq}")
                nc.gpsimd.partition_broadcast(rinv, zrow)
                nc.vector.tensor_tensor(
                    out=os_t[c0][hl * HD:(hl + 1) * HD, j0, :],
                    in0=av[0:HD, :], in1=rinv,
                    op=mybir.AluOpType.mult)

        def emit_wo_group(c0, g):
            tl, dc = g // 2, g % 2
            t = 4 * c0 + tl
            pp = ps_proj.tile([P, 512], F32, tag="pp",
                              name=f"wopp{c0}_{g}_{uq}")
            for m in range(NM):
                nc.tensor.matmul(
                    pp, lhsT=os_t[c0][:, m, tl * P:(tl + 1) * P],
                    rhs=wo_bf[:, m, dc * 512:(dc + 1) * 512],
                    start=(m == 0), stop=(m == NM - 1),
                    skip_group_check=True)
            yt = ypool.tile([P, 512], F32, tag="y",
                            name=f"yt{c0}_{g}_{uq}")
            nc.vector.tensor_copy(out=yt, in_=pp)
            nc.sync.dma_start(
                out=y_out[t * P:(t + 1) * P, dc * 512:(dc + 1) * 512],
                in_=yt)

        prev = None
        wo_pending = []   # (c, next_group_idx)
        for c in range(NCH):
            for j in range(NM):
                exp_pair = [exppool.tile([P, NT, 512], BF16, tag=f"exp{hl}",
                                         name=f"exp{hl}_{c}_{j}_{uq}")
                            for hl in range(2)]
                for kg in range(8):
                    ps_pair = [ps_s.tile([P, 2, 512], F32, tag="psS",
                                         name=f"psS{hl}_{c}_{j}_{kg}_{uq}")
                               for hl in range(2)]
                    for k2 in range(2):
                        ki = kg * 2 + k2
                        for hl in range(2):
                            rows = slice(hl * HD, (hl + 1) * HD)
                            nc.tensor.matmul(
                                ps_pair[hl][:, k2, :],
                                lhsT=kt[rows, j, ki * P:(ki + 1) * P],
                                rhs=qt[c][rows, j, :],
                                start=True, stop=True,
                                skip_group_check=True)
                    for hl in range(2):
                        if _EXP_ON_DVE[0]:
                            nc.vector.tensor_copy(
                                out=exp_pair[hl][:, kg * 2:kg * 2 + 2, :],
                                in_=ps_pair[hl][:, :, :])
                        else:
                            nc.scalar.activation(
                                out=exp_pair[hl][:, kg * 2:kg * 2 + 2, :],
                                in_=ps_pair[hl][:, :, :],
                                func=mybir.ActivationFunctionType.Exp,
                                scale=SCALE)
                    if prev is not None:
                        emit_av_chunk(prev, kg)
                    if wo_pending and kg in (3, 7):
                        c0, g = wo_pending[0]
                        emit_wo_group(c0, g)
                        if g + 1 >= 8:
                            wo_pending.pop(0)
                        else:
                            wo_pending[0] = (c0, g + 1)
                if prev is not None:
                    emit_normalize(prev)
                    if prev[1] == NM - 1:      # finished batch-chunk prev[0]
                        wo_pending.append((prev[0], 0))
                avs = [ps_av.tile([HD + 1, 512], F32, tag="av",
                                  name=f"av{c}_{j}_{hl}_{uq}")
                       for hl in range(2)]
                prev = (c, j, exp_pair, avs)
        # drain: AV + normalize of the last (c,j), then remaining Wo groups
        for kg in range(8):
            emit_av_chunk(prev, kg)
        emit_normalize(prev)
        wo_pending.append((prev[0], 0))
        for c0, g0 in list(wo_pending):
            for g in range(g0, 8):
                emit_wo_group(c0, g)


_CACHE = {}


def _get_exec(ln_affine=True, repeat=1, hw_loop=0):
    """Build the Bass program once and wrap it in a reusable jitted executor."""
    key = ("exec", ln_affine, repeat, hw_loop)
    if key in _CACHE:
        return _CACHE[key]

    import jax
    from jax.sharding import Mesh, PartitionSpec
    from jax.experimental.shard_map import shard_map
    from concourse import bass2jax

    nc = _build_program(ln_affine=ln_affine, repeat=repeat, hw_loop=hw_loop)
    bass2jax.install_neuronx_cc_hook()

    partition_name = (nc.partition_id_tensor.name
                      if nc.partition_id_tensor else None)
    in_names, out_names, out_avals, zero_shapes = [], [], [], []
    for alloc in nc.m.functions[0].allocations:
        if not isinstance(alloc, mybir.MemoryLocationSet):
            continue
        name = alloc.memorylocations[0].name
        if alloc.kind == "ExternalInput":
            if name != partition_name:
                in_names.append(name)
        elif alloc.kind == "ExternalOutput":
            shape = tuple(alloc.tensor_shape)
            dtype = mybir.dt.np(alloc.dtype)
            out_names.append(name)
            out_avals.append(jax.core.ShapedArray(shape, dtype))
            zero_shapes.append((shape, dtype))
    n_params = len(in_names)
    n_outs = len(out_avals)
    all_names = list(in_names) + list(out_names)
    if partition_name is not None:
        all_names.append(partition_name)
    donate = tuple(range(n_params, n_params + n_outs))

    def _body(*args):
        operands = list(args)
        if partition_name is not None:
            operands.append(bass2jax.partition_id_tensor())
        outs = bass2jax._bass_exec_p.bind(
            *operands,
            out_avals=tuple(out_avals),
            in_names=tuple(all_names),
            out_names=tuple(out_names),
            lowering_input_output_aliases=(),
            sim_require_finite=True,
            sim_require_nnan=True,
            nc=nc,
        )
        return tuple(outs)

    n_cores = 8
    devices = jax.devices()[:n_cores]
    mesh = Mesh(np.asarray(devices), ("core",))
    in_specs = (PartitionSpec("core"),) * (n_params + n_outs)
    out_specs = (PartitionSpec("core"),) * n_outs
    sharded = jax.jit(
        shard_map(_body, mesh=mesh, in_specs=in_specs, out_specs=out_specs,
                  check_rep=False),
        donate_argnums=donate, keep_unused=True)

    def execute(in_maps):
        per_core = [[np.ascontiguousarray(np.asarray(m[name], np.float32))
                     for name in in_names] for m in in_maps]
        concat_in = [np.concatenate([per_core[cc][i] for cc in range(n_cores)],
                                    axis=0) for i in range(n_params)]
        concat_zeros = [np.zeros((n_cores * s[0], *s[1:]), d)
                        for (s, d) in zero_shapes]
        out_arrs = sharded(*concat_in, *concat_zeros)
        return [
            {name: np.asarray(out_arrs[i]).reshape(n_cores, *out_avals[i].shape)[cc]
             for i, name in enumerate(out_names)}
            for cc in range(n_cores)
        ]

    _CACHE[key] = execute
    _CACHE[("parts", ln_affine, repeat, hw_loop)] = {
        "sharded": sharded, "in_names": in_names, "n_params": n_params,
        "out_names": out_names, "out_avals": out_avals,
        "zero_shapes": zero_shapes, "mesh": mesh, "n_cores": n_cores,
        "body": _body, "in_specs": in_specs, "out_specs": out_specs,
        "donate": donate,
    }
    return execute


def _time_exec(in_maps, iters=5, ln_affine=True, repeat=1, hw_loop=0):
    """Time the sharded executable with device-resident inputs (seconds)."""
    import time
    import jax
    from jax.sharding import NamedSharding, PartitionSpec

    _get_exec(ln_affine=ln_affine, repeat=repeat, hw_loop=hw_loop)
    parts = _CACHE[("parts", ln_affine, repeat, hw_loop)]
    sharded = parts["sharded"]
    n_cores = parts["n_cores"]
    sh = NamedSharding(parts["mesh"], PartitionSpec("core"))
    per_core = [[np.ascontiguousarray(np.asarray(m[name], np.float32))
                 for name in parts["in_names"]] for m in in_maps]
    concat_in = [np.concatenate([per_core[cc][i] for cc in range(n_cores)],
                                axis=0) for i in range(parts["n_params"])]
    in_dev = [jax.device_put(a, sh) for a in concat_in]
    jax.block_until_ready(in_dev)
    times = []
    for _ in range(iters):
        z_dev = [jax.device_put(
                     np.zeros((n_cores * s[0], *s[1:]), d), sh)
                 for (s, d) in parts["zero_shapes"]]
        jax.block_until_ready(z_dev)
        t0 = time.perf_counter()
        out = sharded(*in_dev, *z_dev)
        jax.block_until_ready(out)
        times.append(time.perf_counter() - t0)
        del out
    return times


def _ln_is_identity(inputs):
    return all(
        np.all(np.asarray(inputs[k], np.float32) == v)
        for k, v in (("gq", 1.0), ("betq", 0.0), ("gkv", 1.0), ("betkv", 0.0))
    )


def _make_in_maps(inputs):
    q = np.asarray(inputs["query"], np.float32)
    c = np.asarray(inputs["context"], np.float32)
    Wq = np.asarray(inputs["Wq"], np.float32)
    Wk = np.asarray(inputs["Wk"], np.float32)
    Wv = np.asarray(inputs["Wv"], np.float32)
    Wo = np.asarray(inputs["Wo"], np.float32)
    bq = np.asarray(inputs["bq"], np.float32)
    bk = np.asarray(inputs["bk"], np.float32)
    bv = np.asarray(inputs["bv"], np.float32)
    gq = np.asarray(inputs["gq"], np.float32)
    btq = np.asarray(inputs["betq"], np.float32)
    gkv = np.asarray(inputs["gkv"], np.float32)
    btkv = np.asarray(inputs["betkv"], np.float32)
    in_maps = []
    for core in range(8):
        b, hg = core // 2, core % 2
        sl = slice(hg * DG, (hg + 1) * DG)
        in_maps.append({
            "q_in": q[b], "c_in": c[b],
            "wq": Wq[:, sl], "wk": Wk[:, sl], "wv": Wv[:, sl],
            "wo": Wo[sl, :],
            "bq": bq[sl], "bk": bk[sl], "bv": bv[sl],
            "gq": gq, "btq": btq, "gkv": gkv, "btkv": btkv,
        })
    return in_maps


def kernel(**inputs):
    execute = _get_exec(ln_affine=not _ln_is_identity(inputs))
    in_maps = _make_in_maps(inputs)
    results = execute(in_maps)
    bo = np.asarray(inputs["bo"], np.float32)
    B = 4
    out = np.empty((B, N_TOK, D), np.float32)
    for b in range(B):
        out[b] = results[2 * b]["y_out"] + results[2 * b + 1]["y_out"] + bo
    return out


def _dma_only(nc, tc, q_in, c_in, wq, wk, wv, wo, y_out):
    mode = _DMA_MODE[0]
    with tc.tile_pool(name="dml", bufs=4) as pool:
        acc = pool.tile([P, D], F32, tag="acc")
        if mode == 2:
            for src in (q_in, c_in):
                r4 = src.rearrange("(n i p) d -> n p i d", p=P, i=4)
                for n in range(NT // 4):
                    xb = pool.tile([P, 4, D], F32, tag="dxb")
                    eng = nc.sync if n % 2 == 0 else nc.scalar
                    eng.dma_start(out=xb, in_=r4[n])
                    nc.vector.tensor_copy(out=acc, in_=xb[:, 0, :])
        else:
            for src in (q_in, c_in):
                for t in range(NT):
                    x = pool.tile([P, D], F32, tag="dx")
                    eng = nc.sync if (mode == 0 or t % 2 == 0) else nc.scalar
                    eng.dma_start(out=x, in_=src[t * P:(t + 1) * P, :])
                    nc.vector.tensor_copy(out=acc, in_=x)
        for w_d in (wq, wk, wv):
            w_r = w_d.rearrange("(s p) n -> p s n", p=P)
            for s_ in range(KS):
                x = pool.tile([P, D], F32, tag="dx")
                nc.sync.dma_start(out=x[:, :DG], in_=w_r[:, s_, :])
                nc.vector.tensor_copy(out=acc, in_=x)
        wo_r = wo.rearrange("(m p) n -> p m n", p=P)
        for m in range(NM):
            x = pool.tile([P, D], F32, tag="dx")
            nc.sync.dma_start(out=x, in_=wo_r[:, m, :])
            nc.vector.tensor_copy(out=acc, in_=x)
        nc.sync.dma_start(out=y_out[0:P, 0:D], in_=acc)


# revision 31
# speedup vs baseline: 1.0298x; 1.0133x over previous
"""Trainium2 Bass kernel for nn_CrossAttention (B=4, Nq=Nk=2048, D=1024, H=16).

Sharding: 8 cores = (batch b in 0..3) x (head-group hg in 0..1), 8 heads/core.
Each core gets its batch's query/context plus the column slice of Wq/Wk/Wv and
row slice of Wo for its 8 heads; LayerNorm params are replicated.  Host sums
the two head-group partial outputs per batch and adds bo.

Per-core pipeline (all matmuls bf16 with fp32 PSUM accumulation):
  LN (fp32, bn_stats) -> PE transpose (bf16) -> Q/K/V projections
  (Q,K in [dout, tok] layout; V in [tok, dout] with a ones column appended)
  -> S^T = K Q^T tiles (2 heads packed in the 128-row PE array)
  -> exp on ScalarE with the 1/sqrt(dh) scale folded in
  -> AV matmul with M=65 (row 64 = softmax denominator Z, for free)
  -> normalize via GPSIMD partition_broadcast of 1/Z + DVE multiply
  -> Wo row-slice matmul -> fp32 partial output.
"""

import numpy as np

import concourse.bass as bass
import concourse.mybir as mybir
import concourse.tile as tile
from concourse import bacc
from concourse.masks import make_identity

P = 128
N_TOK = 2048          # tokens per batch (both Nq and Nk)
D = 1024              # model dim
KS = D // P           # 8 contraction subtiles
DG = 512              # per-core projection width (8 heads * 64)
NM = DG // P          # 4 output blocks / head-pair groups
NH = 8                # heads per core
HD = 64
NT = N_TOK // P       # 16 token tiles
NCH = N_TOK // 512    # 4 token chunks of 512
SCALE = HD ** -0.5
EPS = 1e-5

F32 = mybir.dt.float32
BF16 = mybir.dt.bfloat16
_UNIQ = [0]
_EXP_ON_DVE = [False]  # timing-diagnostic only: wrong numerics
_PHASE_ONLY = [None]   # None | "proj" | "dma"  (timing-diagnostic only)
_DMA_MODE = [0]


def _build_program(ln_affine=True, repeat=1, hw_loop=0):
    nc = bacc.Bacc("TRN2", target_bir_lowering=False, debug=False)

    q_in = nc.dram_tensor("q_in", (N_TOK, D), F32, kind="ExternalInput")
    c_in = nc.dram_tensor("c_in", (N_TOK, D), F32, kind="ExternalInput")
    wq = nc.dram_tensor("wq", (D, DG), F32, kind="ExternalInput")
    wk = nc.dram_tensor("wk", (D, DG), F32, kind="ExternalInput")
    wv = nc.dram_tensor("wv", (D, DG), F32, kind="ExternalInput")
    wo = nc.dram_tensor("wo", (DG, D), F32, kind="ExternalInput")
    bq_d = nc.dram_tensor("bq", (DG,), F32, kind="ExternalInput")
    bk_d = nc.dram_tensor("bk", (DG,), F32, kind="ExternalInput")
    bv_d = nc.dram_tensor("bv", (DG,), F32, kind="ExternalInput")
    gq_d = nc.dram_tensor("gq", (D,), F32, kind="ExternalInput")
    btq_d = nc.dram_tensor("btq", (D,), F32, kind="ExternalInput")
    gkv_d = nc.dram_tensor("gkv", (D,), F32, kind="ExternalInput")
    btkv_d = nc.dram_tensor("btkv", (D,), F32, kind="ExternalInput")
    y_out = nc.dram_tensor("y_out", (N_TOK, D), F32, kind="ExternalOutput")

    import contextlib

    with tile.TileContext(nc) as tc:
        loop_ctx = tc.For_i(0, hw_loop, 1) if hw_loop else None
        with (loop_ctx if loop_ctx is not None else contextlib.nullcontext()):
         for _rep in range(repeat):
            _UNIQ[0] += 1
            with (
                tc.tile_pool(name="persist", bufs=1) as persist,
                tc.tile_pool(name="wopool", bufs=1) as wopool,
                tc.tile_pool(name="ps_proj", bufs=2, space="PSUM") as ps_proj,
            ):
                # ---------------- persistent tensors ----------------
                qt = [persist.tile([P, NM, 512], BF16, tag=f"qt{c}",
                                   name=f"qt{c}_{_UNIQ[0]}")
                      for c in range(NCH)]   # Q^T per token chunk
                kt = persist.tile([P, NM, N_TOK], BF16, tag="kt")   # K^T
                vs = persist.tile([P, NT, NH, HD + 1], BF16, tag="vs")
                os_t = [persist.tile([P, NM, 512], BF16, tag=f"os{c}",
                                     name=f"os{c}_{_UNIQ[0]}")
                        for c in range(NCH)]
                nc.vector.memset(vs[:, :, :, HD:HD + 1], 1.0)
                wo_bf = wopool.tile([P, NM, D], BF16, tag="wo_bf")

                if _PHASE_ONLY[0] == "dma":
                    _dma_only(nc, tc, q_in, c_in, wq, wk, wv, wo, y_out)
                    continue
                _proj_phase(nc, tc, q_in, c_in, wq, wk, wv, wo,
                            bq_d, bk_d, bv_d, gq_d, btq_d, gkv_d, btkv_d,
                            qt, kt, vs, wo_bf, ps_proj, ln_affine)
                if _PHASE_ONLY[0] == "proj":
                    yt0 = persist.tile([P, 512], F32, tag="y0",
                                       name=f"y0_{_UNIQ[0]}")
                    nc.vector.tensor_copy(out=yt0[:, :],
                                          in_=kt[:, 0, 0:512])
                    nc.sync.dma_start(out=y_out[0:P, 0:512], in_=yt0)
                    continue
                _attn_phase(nc, tc, qt, kt, vs, os_t, wo_bf, ps_proj, y_out)

    nc.finalize()
    return nc


def _proj_phase(nc, tc, q_in, c_in, wq, wk, wv, wo,
                bq_d, bk_d, bv_d, gq_d, btq_d, gkv_d, btkv_d,
                qt, kt, vs, wo_bf, ps_proj, ln_affine):
    with (
        tc.tile_pool(name="consts", bufs=1) as consts,
        tc.tile_pool(name="weights", bufs=1) as wpool,
        tc.tile_pool(name="wstage", bufs=2) as wstage,
        tc.tile_pool(name="xload", bufs=2) as xpool,
        tc.tile_pool(name="stats", bufs=4) as stats,
        tc.tile_pool(name="lntmp", bufs=2) as lntmp,
        tc.tile_pool(name="lnout", bufs=5) as lnpool,
        tc.tile_pool(name="lnT", bufs=2) as lntpool,
        tc.tile_pool(name="ps_tr", bufs=2, space="PSUM") as ps_tr,
    ):
            # ---------------- constants ----------------
            ident = consts.tile([P, P], BF16, tag="ident")
            make_identity(nc, ident)
            eps_t = consts.tile([P, 1], F32, tag="eps")
            nc.vector.memset(eps_t, EPS)
            if ln_affine:
                gq_b = consts.tile([P, D], F32, tag="gq_b")
                nc.gpsimd.dma_start(out=gq_b,
                                    in_=gq_d[None, :].to_broadcast((P, D)))
                btq_b = consts.tile([P, D], F32, tag="btq_b")
                nc.gpsimd.dma_start(out=btq_b,
                                    in_=btq_d[None, :].to_broadcast((P, D)))
                gkv_b = consts.tile([P, D], F32, tag="gkv_b")
                nc.gpsimd.dma_start(out=gkv_b,
                                    in_=gkv_d[None, :].to_broadcast((P, D)))
                btkv_b = consts.tile([P, D], F32, tag="btkv_b")
                nc.gpsimd.dma_start(out=btkv_b,
                                    in_=btkv_d[None, :].to_broadcast((P, D)))
            else:
                gq_b = btq_b = gkv_b = btkv_b = None
            bv_b = consts.tile([P, DG], F32, tag="bv_b")
            nc.gpsimd.dma_start(out=bv_b, in_=bv_d[None, :].to_broadcast((P, DG)))
            bq_c = consts.tile([P, NM], F32, tag="bq_c")
            nc.sync.dma_start(out=bq_c, in_=bq_d.rearrange("(m p) -> p m", p=P))
            bk_c = consts.tile([P, NM], F32, tag="bk_c")
            nc.sync.dma_start(out=bk_c, in_=bk_d.rearrange("(m p) -> p m", p=P))

            # ---------------- weights (fp32 -> bf16) ----------------
            wq_bf = wpool.tile([P, KS, DG], BF16, tag="wq_bf")
            wk_bf = wpool.tile([P, KS, DG], BF16, tag="wk_bf")
            wv_bf = wpool.tile([P, KS, DG], BF16, tag="wv_bf")
            for wi, (w_d, w_bf) in enumerate(
                    ((wq, wq_bf), (wk, wk_bf), (wv, wv_bf))):
                w_r = w_d.rearrange("(a s p) n -> p a s n", p=P, s=4)
                for a in range(2):
                    st = wstage.tile([P, 4, DG], F32, tag="wst")
                    (nc.sync if (wi + a) % 2 == 0 else nc.scalar).dma_start(
                        out=st, in_=w_r[:, a])
                    nc.vector.tensor_copy(
                        out=w_bf[:, a * 4:(a + 1) * 4, :], in_=st)
            wo_r = wo.rearrange("(m p) n -> p m n", p=P)
            st = wstage.tile([P, 4, DG], F32, tag="wst")
            st2 = wstage.tile([P, 4, DG], F32, tag="wst")
            nc.sync.dma_start(out=st, in_=wo_r[:, :, 0:DG])
            nc.scalar.dma_start(out=st2, in_=wo_r[:, :, DG:D])
            nc.vector.tensor_copy(out=wo_bf[:, :, 0:DG], in_=st)
            nc.vector.tensor_copy(out=wo_bf[:, :, DG:D], in_=st2)

            # ---------------- LN + transpose + projections ----------------
            def ln_tile(xb, tl, g_b, b_b):
                """LayerNorm token tile xb[:, tl, :] -> [128, 1024] bf16."""
                x = xb[:, tl, :]
                st = stats.tile([P, 2, 6], F32, tag="bnst")
                nc.vector.bn_stats(out=st[:, 0, :], in_=x[:, 0:512])
                nc.vector.bn_stats(out=st[:, 1, :], in_=x[:, 512:1024])
                mv = stats.tile([P, 2], F32, tag="mv")
                nc.vector.bn_aggr(out=mv, in_=st)
                lnv = stats.tile([P, 1], F32, tag="lnv")
                nc.scalar.activation(out=lnv, in_=mv[:, 1:2],
                                     func=mybir.ActivationFunctionType.Ln,
                                     bias=eps_t)
                rstd = stats.tile([P, 1], F32, tag="rstd")
                nc.scalar.activation(out=rstd, in_=lnv,
                                     func=mybir.ActivationFunctionType.Exp,
                                     scale=-0.5)
                lnt = lnpool.tile([P, D], BF16, tag="ln")
                if not ln_affine:
                    nc.vector.tensor_scalar(out=lnt, in0=x, scalar1=mv[:, 0:1],
                                            scalar2=rstd,
                                            op0=mybir.AluOpType.subtract,
                                            op1=mybir.AluOpType.mult)
                    return lnt
                xc = lntmp.tile([P, D], F32, tag="xc")
                nc.vector.tensor_scalar(out=xc, in0=x, scalar1=mv[:, 0:1],
                                        scalar2=rstd,
                                        op0=mybir.AluOpType.subtract,
                                        op1=mybir.AluOpType.mult)
                xg = lntmp.tile([P, D], F32, tag="xg")
                nc.vector.tensor_tensor(out=xg, in0=xc, in1=g_b,
                                        op=mybir.AluOpType.mult)
                nc.vector.tensor_tensor(out=lnt, in0=xg, in1=b_b,
                                        op=mybir.AluOpType.add)
                return lnt

            def transpose_chunk(ln_tiles):
                """4 LN tiles ([128 tok, 1024 feat]) -> lnT [128 feat, 8, 512 tok]."""
                lnT = lntpool.tile([P, KS, 512], BF16, tag="lnT")
                for s in range(KS):
                    pt = ps_tr.tile([P, 512], BF16, tag="tr")
                    for tl in range(4):
                        nc.tensor.transpose(pt[:, tl * P:(tl + 1) * P],
                                            ln_tiles[tl][:, s * P:(s + 1) * P],
                                            ident)
                    nc.vector.tensor_copy(out=lnT[:, s, :], in_=pt)
                return lnT

            # context: K^T, V
            c_r = c_in.rearrange("(n i p) d -> n p i d", p=P, i=4)
            q_r = q_in.rearrange("(n i p) d -> n p i d", p=P, i=4)
            for c in range(NCH):
                xb = xpool.tile([P, 4, D], F32, tag="xb")
                (nc.sync if c % 2 == 0 else nc.scalar).dma_start(
                    out=xb, in_=c_r[c])
                ln_tiles = [ln_tile(xb, tl, gkv_b, btkv_b)
                            for tl in range(4)]
                lnT = transpose_chunk(ln_tiles)
                for m in range(NM):
                    pp = ps_proj.tile([P, 512], F32, tag="pp")
                    for s in range(KS):
                        nc.tensor.matmul(pp, lhsT=wk_bf[:, s, m * P:(m + 1) * P],
                                         rhs=lnT[:, s, :],
                                         start=(s == 0), stop=(s == KS - 1))
                    nc.vector.tensor_scalar_add(
                        out=kt[:, m, c * 512:(c + 1) * 512], in0=pp,
                        scalar1=bk_c[:, m:m + 1])
                for tl in range(4):
                    t = 4 * c + tl
                    pp = ps_proj.tile([P, 512], F32, tag="pp")
                    for s in range(KS):
                        nc.tensor.matmul(pp, lhsT=lnT[:, s, tl * P:(tl + 1) * P],
                                         rhs=wv_bf[:, s, :],
                                         start=(s == 0), stop=(s == KS - 1))
                    nc.vector.tensor_tensor(
                        out=vs[:, t, :, 0:HD],
                        in0=pp.rearrange("p (h d) -> p h d", h=NH),
                        in1=bv_b.rearrange("p (h d) -> p h d", h=NH),
                        op=mybir.AluOpType.add)

            # query: Q^T
            for c in range(NCH):
                xb = xpool.tile([P, 4, D], F32, tag="xb")
                (nc.sync if c % 2 == 0 else nc.scalar).dma_start(
                    out=xb, in_=q_r[c])
                ln_tiles = [ln_tile(xb, tl, gq_b, btq_b)
                            for tl in range(4)]
                lnT = transpose_chunk(ln_tiles)
                for m in range(NM):
                    pp = ps_proj.tile([P, 512], F32, tag="pp")
                    for s in range(KS):
                        nc.tensor.matmul(pp, lhsT=wq_bf[:, s, m * P:(m + 1) * P],
                                         rhs=lnT[:, s, :],
                                         start=(s == 0), stop=(s == KS - 1))
                    nc.vector.tensor_scalar_add(
                        out=qt[c][:, m, :], in0=pp,
                        scalar1=bq_c[:, m:m + 1])


def _attn_phase(nc, tc, qt, kt, vs, os_t, wo_bf, ps_proj, y_out):
    with (
        tc.tile_pool(name="exp", bufs=3) as exppool,
        tc.tile_pool(name="smalls", bufs=2) as smalls,
        tc.tile_pool(name="yout", bufs=3) as ypool,
        tc.tile_pool(name="ps_s", bufs=2, space="PSUM") as ps_s,
        tc.tile_pool(name="ps_av", bufs=2, space="PSUM") as ps_av,
    ):
        uq = _UNIQ[0]

        def emit_av_chunk(prev, kg):
            c0, j0, exp_pair, avs = prev
            for hl in range(2):
                for k2 in range(2):
                    ki = kg * 2 + k2
                    nc.tensor.matmul(avs[hl], lhsT=vs[:, ki, 2 * j0 + hl, :],
                                     rhs=exp_pair[hl][:, ki, :],
                                     start=(ki == 0), stop=(ki == NT - 1),
                                     skip_group_check=True)

        def emit_normalize(prev):
            c0, j0, exp_pair, avs = prev
            for hl in range(2):
                av = avs[hl]
                zrow = smalls.tile([1, 512], F32, tag="zrow",
                                   name=f"zrow{c0}_{j0}_{hl}_{uq}")
                nc.vector.reciprocal(out=zrow, in_=av[HD:HD + 1, :])
                rinv = smalls.tile([HD, 512], F32, tag="rinv",
                                   name=f"rinv{c0}_{j0}_{hl}_{uq}")
                nc.gpsimd.partition_broadcast(rinv, zrow)
                nc.vector.tensor_tensor(
                    out=os_t[c0][hl * HD:(hl + 1) * HD, j0, :],
                    in0=av[0:HD, :], in1=rinv,
                    op=mybir.AluOpType.mult)

        yt_cur = [None]

        def emit_wo_group(c0, g):
            tl, dc = g // 2, g % 2
            t = 4 * c0 + tl
            pp = ps_proj.tile([P, 512], F32, tag="pp",
                              name=f"wopp{c0}_{g}_{uq}")
            for m in range(NM):
                nc.tensor.matmul(
                    pp, lhsT=os_t[c0][:, m, tl * P:(tl + 1) * P],
                    rhs=wo_bf[:, m, dc * 512:(dc + 1) * 512],
                    start=(m == 0), stop=(m == NM - 1),
                    skip_group_check=True)
            if dc == 0:
                yt_cur[0] = ypool.tile([P, 2, 512], F32, tag="y",
                                       name=f"yt{c0}_{g}_{uq}")
            nc.vector.tensor_copy(out=yt_cur[0][:, dc, :], in_=pp)
            if dc == 1:
                nc.sync.dma_start(
                    out=y_out[t * P:(t + 1) * P, :], in_=yt_cur[0])

        prev = None
        wo_pending = []   # (c, next_group_idx)
        for c in range(NCH):
            for j in range(NM):
                exp_pair = [exppool.tile([P, NT, 512], BF16, tag=f"exp{hl}",
                                         name=f"exp{hl}_{c}_{j}_{uq}")
                            for hl in range(2)]
                for kg in range(8):
                    ps_pair = [ps_s.tile([P, 2, 512], F32, tag="psS",
                                         name=f"psS{hl}_{c}_{j}_{kg}_{uq}")
                               for hl in range(2)]
                    for k2 in range(2):
                        ki = kg * 2 + k2
                        for hl in range(2):
                            rows = slice(hl * HD, (hl + 1) * HD)
                            nc.tensor.matmul(
                                ps_pair[hl][:, k2, :],
                                lhsT=kt[rows, j, ki * P:(ki + 1) * P],
                                rhs=qt[c][rows, j, :],
                                start=True, stop=True,
                                skip_group_check=True)
                    for hl in range(2):
                        if _EXP_ON_DVE[0]:
                            nc.vector.tensor_copy(
                                out=exp_pair[hl][:, kg * 2:kg * 2 + 2, :],
                                in_=ps_pair[hl][:, :, :])
                        else:
                            nc.scalar.activation(
                                out=exp_pair[hl][:, kg * 2:kg * 2 + 2, :],
                                in_=ps_pair[hl][:, :, :],
                                func=mybir.ActivationFunctionType.Exp,
                                scale=SCALE)
                    if prev is not None:
                        emit_av_chunk(prev, kg)
                    if wo_pending and kg in (3, 7):
                        c0, g = wo_pending[0]
                        emit_wo_group(c0, g)
                        if g + 1 >= 8:
                            wo_pending.pop(0)
                        else:
                            wo_pending[0] = (c0, g + 1)
                if prev is not None:
                    emit_normalize(prev)
                    if prev[1] == NM - 1:      # finished batch-chunk prev[0]
                        wo_pending.append((prev[0], 0))
                avs = [ps_av.tile([HD + 1, 512], F32, tag="av",
                                  name=f"av{c}_{j}_{hl}_{uq}")
                       for hl in range(2)]
                prev = (c, j, exp_pair, avs)
        # drain: AV + normalize of the last (c,j), then remaining Wo groups
        for kg in range(8):
            emit_av_chunk(prev, kg)
        emit_normalize(prev)
        wo_pending.append((prev[0], 0))
        for c0, g0 in list(wo_pending):
            for g in range(g0, 8):
                emit_wo_group(c0, g)


_CACHE = {}


def _get_exec(ln_affine=True, repeat=1, hw_loop=0):
    """Build the Bass program once and wrap it in a reusable jitted executor."""
    key = ("exec", ln_affine, repeat, hw_loop)
    if key in _CACHE:
        return _CACHE[key]

    import jax
    from jax.sharding import Mesh, PartitionSpec
    from jax.experimental.shard_map import shard_map
    from concourse import bass2jax

    nc = _build_program(ln_affine=ln_affine, repeat=repeat, hw_loop=hw_loop)
    bass2jax.install_neuronx_cc_hook()

    partition_name = (nc.partition_id_tensor.name
                      if nc.partition_id_tensor else None)
    in_names, out_names, out_avals, zero_shapes = [], [], [], []
    for alloc in nc.m.functions[0].allocations:
        if not isinstance(alloc, mybir.MemoryLocationSet):
            continue
        name = alloc.memorylocations[0].name
        if alloc.kind == "ExternalInput":
            if name != partition_name:
                in_names.append(name)
        elif alloc.kind == "ExternalOutput":
            shape = tuple(alloc.tensor_shape)
            dtype = mybir.dt.np(alloc.dtype)
            out_names.append(name)
            out_avals.append(jax.core.ShapedArray(shape, dtype))
            zero_shapes.append((shape, dtype))
    n_params = len(in_names)
    n_outs = len(out_avals)
    all_names = list(in_names) + list(out_names)
    if partition_name is not None:
        all_names.append(partition_name)
    donate = tuple(range(n_params, n_params + n_outs))

    def _body(*args):
        operands = list(args)
        if partition_name is not None:
            operands.append(bass2jax.partition_id_tensor())
        outs = bass2jax._bass_exec_p.bind(
            *operands,
            out_avals=tuple(out_avals),
            in_names=tuple(all_names),
            out_names=tuple(out_names),
            lowering_input_output_aliases=(),
            sim_require_finite=True,
            sim_require_nnan=True,
            nc=nc,
        )
        return tuple(outs)

    n_cores = 8
    devices = jax.devices()[:n_cores]
    mesh = Mesh(np.asarray(devices), ("core",))
    in_specs = (PartitionSpec("core"),) * (n_params + n_outs)
    out_specs = (PartitionSpec("core"),) * n_outs
    sharded = jax.jit(
        shard_map(_body, mesh=mesh, in_specs=in_specs, out_specs=out_specs,
                  check_rep=False),
        donate_argnums=donate, keep_unused=True)

    def execute(in_maps):
        per_core = [[np.ascontiguousarray(np.asarray(m[name], np.float32))
                     for name in in_names] for m in in_maps]
        concat_in = [np.concatenate([per_core[cc][i] for cc in range(n_cores)],
                                    axis=0) for i in range(n_params)]
        concat_zeros = [np.zeros((n_cores * s[0], *s[1:]), d)
                        for (s, d) in zero_shapes]
        out_arrs = sharded(*concat_in, *concat_zeros)
        return [
            {name: np.asarray(out_arrs[i]).reshape(n_cores, *out_avals[i].shape)[cc]
             for i, name in enumerate(out_names)}
            for cc in range(n_cores)
        ]

    _CACHE[key] = execute
    _CACHE[("parts", ln_affine, repeat, hw_loop)] = {
        "sharded": sharded, "in_names": in_names, "n_params": n_params,
        "out_names": out_names, "out_avals": out_avals,
        "zero_shapes": zero_shapes, "mesh": mesh, "n_cores": n_cores,
        "body": _body, "in_specs": in_specs, "out_specs": out_specs,
        "donate": donate,
    }
    return execute


def _time_exec(in_maps, iters=5, ln_affine=True, repeat=1, hw_loop=0):
    """Time the sharded executable with device-resident inputs (seconds)."""
    import time
    import jax
    from jax.sharding import NamedSharding, PartitionSpec

    _get_exec(ln_affine=ln_affine, repeat=repeat, hw_loop=hw_loop)
    parts = _CACHE[("parts", ln_affine, repeat, hw_loop)]
    sharded = parts["sharded"]
    n_cores = parts["n_cores"]
    sh = NamedSharding(parts["mesh"], PartitionSpec("core"))
    per_core = [[np.ascontiguousarray(np.asarray(m[name], np.float32))
                 for name in parts["in_names"]] for m in in_maps]
    concat_in = [np.concatenate([per_core[cc][i] for cc in range(n_cores)],
                                axis=0) for i in range(parts["n_params"])]
    in_dev = [jax.device_put(a, sh) for a in concat_in]
    jax.block_until_ready(in_dev)
    times = []
    for _ in range(iters):
        z_dev = [jax.device_put(
                     np.zeros((n_cores * s[0], *s[1:]), d), sh)
                 for (s, d) in parts["zero_shapes"]]
        jax.block_until_ready(z_dev)
        t0 = time.perf_counter()
        out = sharded(*in_dev, *z_dev)
        jax.block_until_ready(out)
        times.append(time.perf_counter() - t0)
        del out
    return times


def _ln_is_identity(inputs):
    return all(
        np.all(np.asarray(inputs[k], np.float32) == v)
        for k, v in (("gq", 1.0), ("betq", 0.0), ("gkv", 1.0), ("betkv", 0.0))
    )


def _make_in_maps(inputs):
    q = np.asarray(inputs["query"], np.float32)
    c = np.asarray(inputs["context"], np.float32)
    Wq = np.asarray(inputs["Wq"], np.float32)
    Wk = np.asarray(inputs["Wk"], np.float32)
    Wv = np.asarray(inputs["Wv"], np.float32)
    Wo = np.asarray(inputs["Wo"], np.float32)
    bq = np.asarray(inputs["bq"], np.float32)
    bk = np.asarray(inputs["bk"], np.float32)
    bv = np.asarray(inputs["bv"], np.float32)
    gq = np.asarray(inputs["gq"], np.float32)
    btq = np.asarray(inputs["betq"], np.float32)
    gkv = np.asarray(inputs["gkv"], np.float32)
    btkv = np.asarray(inputs["betkv"], np.float32)
    in_maps = []
    for core in range(8):
        b, hg = core // 2, core % 2
        sl = slice(hg * DG, (hg + 1) * DG)
        in_maps.append({
            "q_in": q[b], "c_in": c[b],
            "wq": Wq[:, sl], "wk": Wk[:, sl], "wv": Wv[:, sl],
            "wo": Wo[sl, :],
            "bq": bq[sl], "bk": bk[sl], "bv": bv[sl],
            "gq": gq, "btq": btq, "gkv": gkv, "btkv": btkv,
        })
    return in_maps


def kernel(**inputs):
    execute = _get_exec(ln_affine=not _ln_is_identity(inputs))
    in_maps = _make_in_maps(inputs)
    results = execute(in_maps)
    bo = np.asarray(inputs["bo"], np.float32)
    B = 4
    out = np.empty((B, N_TOK, D), np.float32)
    for b in range(B):
        out[b] = results[2 * b]["y_out"] + results[2 * b + 1]["y_out"] + bo
    return out


def _dma_only(nc, tc, q_in, c_in, wq, wk, wv, wo, y_out):
    mode = _DMA_MODE[0]
    with tc.tile_pool(name="dml", bufs=4) as pool:
        acc = pool.tile([P, D], F32, tag="acc")
        if mode == 2:
            for src in (q_in, c_in):
                r4 = src.rearrange("(n i p) d -> n p i d", p=P, i=4)
                for n in range(NT // 4):
                    xb = pool.tile([P, 4, D], F32, tag="dxb")
                    eng = nc.sync if n % 2 == 0 else nc.scalar
                    eng.dma_start(out=xb, in_=r4[n])
                    nc.vector.tensor_copy(out=acc, in_=xb[:, 0, :])
        else:
            for src in (q_in, c_in):
                for t in range(NT):
                    x = pool.tile([P, D], F32, tag="dx")
                    eng = nc.sync if (mode == 0 or t % 2 == 0) else nc.scalar
                    eng.dma_start(out=x, in_=src[t * P:(t + 1) * P, :])
                    nc.vector.tensor_copy(out=acc, in_=x)
        for w_d in (wq, wk, wv):
            w_r = w_d.rearrange("(s p) n -> p s n", p=P)
            for s_ in range(KS):
                x = pool.tile([P, D], F32, tag="dx")
                nc.sync.dma_start(out=x[:, :DG], in_=w_r[:, s_, :])
                nc.vector.tensor_copy(out=acc, in_=x)
        wo_r = wo.rearrange("(m p) n -> p m n", p=P)
        for m in range(NM):
            x = pool.tile([P, D], F32, tag="dx")
            nc.sync.dma_start(out=x, in_=wo_r[:, m, :])
            nc.vector.tensor_copy(out=acc, in_=x)
        nc.sync.dma_start(out=y_out[0:P, 0:D], in_=acc)


# revision 32
# speedup vs baseline: 1.0893x; 1.0578x over previous
"""Trainium2 Bass kernel for nn_CrossAttention (B=4, Nq=Nk=2048, D=1024, H=16).

Sharding: 8 cores = (batch b in 0..3) x (head-group hg in 0..1), 8 heads/core.
Each core gets its batch's query/context plus the column slice of Wq/Wk/Wv and
row slice of Wo for its 8 heads; LayerNorm params are replicated.  Host sums
the two head-group partial outputs per batch and adds bo.

Per-core pipeline (all matmuls bf16 with fp32 PSUM accumulation):
  LN (fp32, bn_stats) -> PE transpose (bf16) -> Q/K/V projections
  (Q,K in [dout, tok] layout; V in [tok, dout] with a ones column appended)
  -> S^T = K Q^T tiles (2 heads packed in the 128-row PE array)
  -> exp on ScalarE with the 1/sqrt(dh) scale folded in
  -> AV matmul with M=65 (row 64 = softmax denominator Z, for free)
  -> normalize via GPSIMD partition_broadcast of 1/Z + DVE multiply
  -> Wo row-slice matmul -> fp32 partial output.
"""

import numpy as np

import concourse.bass as bass
import concourse.mybir as mybir
import concourse.tile as tile
from concourse import bacc
from concourse.masks import make_identity

P = 128
N_TOK = 2048          # tokens per batch (both Nq and Nk)
D = 1024              # model dim
KS = D // P           # 8 contraction subtiles
DG = 512              # per-core projection width (8 heads * 64)
NM = DG // P          # 4 output blocks / head-pair groups
NH = 8                # heads per core
HD = 64
NT = N_TOK // P       # 16 token tiles
NCH = N_TOK // 512    # 4 token chunks of 512
SCALE = HD ** -0.5
EPS = 1e-5

F32 = mybir.dt.float32
BF16 = mybir.dt.bfloat16
_UNIQ = [0]
_EXP_ON_DVE = [False]  # timing-diagnostic only: wrong numerics
_PHASE_ONLY = [None]   # None | "proj" | "dma"  (timing-diagnostic only)
_DMA_MODE = [0]


def _build_program(ln_affine=True, repeat=1, hw_loop=0):
    nc = bacc.Bacc("TRN2", target_bir_lowering=False, debug=False)

    q_in = nc.dram_tensor("q_in", (N_TOK, D), F32, kind="ExternalInput")
    c_in = nc.dram_tensor("c_in", (N_TOK, D), F32, kind="ExternalInput")
    wq = nc.dram_tensor("wq", (D, DG), F32, kind="ExternalInput")
    wk = nc.dram_tensor("wk", (D, DG), F32, kind="ExternalInput")
    wv = nc.dram_tensor("wv", (D, DG), F32, kind="ExternalInput")
    wo = nc.dram_tensor("wo", (DG, D), F32, kind="ExternalInput")
    bq_d = nc.dram_tensor("bq", (DG,), F32, kind="ExternalInput")
    bk_d = nc.dram_tensor("bk", (DG,), F32, kind="ExternalInput")
    bv_d = nc.dram_tensor("bv", (DG,), F32, kind="ExternalInput")
    gq_d = nc.dram_tensor("gq", (D,), F32, kind="ExternalInput")
    btq_d = nc.dram_tensor("btq", (D,), F32, kind="ExternalInput")
    gkv_d = nc.dram_tensor("gkv", (D,), F32, kind="ExternalInput")
    btkv_d = nc.dram_tensor("btkv", (D,), F32, kind="ExternalInput")
    y_out = nc.dram_tensor("y_out", (N_TOK, D), F32, kind="ExternalOutput")

    import contextlib

    with tile.TileContext(nc) as tc:
        loop_ctx = tc.For_i(0, hw_loop, 1) if hw_loop else None
        with (loop_ctx if loop_ctx is not None else contextlib.nullcontext()):
         for _rep in range(repeat):
            _UNIQ[0] += 1
            with (
                tc.tile_pool(name="persist", bufs=1) as persist,
                tc.tile_pool(name="wopool", bufs=1) as wopool,
                tc.tile_pool(name="ps_proj", bufs=2, space="PSUM") as ps_proj,
            ):
                # ---------------- persistent tensors ----------------
                qt = [persist.tile([P, NM, 512], BF16, tag=f"qt{c}",
                                   name=f"qt{c}_{_UNIQ[0]}")
                      for c in range(NCH)]   # Q^T per token chunk
                kt = [persist.tile([P, NM, 512], BF16, tag=f"kt{c}",
                                   name=f"kt{c}_{_UNIQ[0]}")
                      for c in range(NCH)]   # K^T per key chunk
                vs = persist.tile([P, NT, NH, HD + 1], BF16, tag="vs")
                os_t = [persist.tile([P, NM, 512], BF16, tag=f"os{c}",
                                     name=f"os{c}_{_UNIQ[0]}")
                        for c in range(NCH)]
                nc.vector.memset(vs[:, :, :, HD:HD + 1], 1.0)
                wo_bf = wopool.tile([P, NM, D], BF16, tag="wo_bf")

                if _PHASE_ONLY[0] == "dma":
                    _dma_only(nc, tc, q_in, c_in, wq, wk, wv, wo, y_out)
                    continue
                _proj_phase(nc, tc, q_in, c_in, wq, wk, wv, wo,
                            bq_d, bk_d, bv_d, gq_d, btq_d, gkv_d, btkv_d,
                            qt, kt, vs, wo_bf, ps_proj, ln_affine)
                if _PHASE_ONLY[0] == "proj":
                    yt0 = persist.tile([P, 512], F32, tag="y0",
                                       name=f"y0_{_UNIQ[0]}")
                    nc.vector.tensor_copy(out=yt0[:, :],
                                          in_=kt[0][:, 0, :])
                    nc.sync.dma_start(out=y_out[0:P, 0:512], in_=yt0)
                    continue
                _attn_phase(nc, tc, qt, kt, vs, os_t, wo_bf, ps_proj, y_out)

    nc.finalize()
    return nc


def _proj_phase(nc, tc, q_in, c_in, wq, wk, wv, wo,
                bq_d, bk_d, bv_d, gq_d, btq_d, gkv_d, btkv_d,
                qt, kt, vs, wo_bf, ps_proj, ln_affine):
    with (
        tc.tile_pool(name="consts", bufs=1) as consts,
        tc.tile_pool(name="weights", bufs=1) as wpool,
        tc.tile_pool(name="wstage", bufs=2) as wstage,
        tc.tile_pool(name="xload", bufs=2) as xpool,
        tc.tile_pool(name="stats", bufs=4) as stats,
        tc.tile_pool(name="lntmp", bufs=2) as lntmp,
        tc.tile_pool(name="lnout", bufs=5) as lnpool,
        tc.tile_pool(name="lnT", bufs=2) as lntpool,
        tc.tile_pool(name="ps_tr", bufs=2, space="PSUM") as ps_tr,
    ):
            # ---------------- constants ----------------
            ident = consts.tile([P, P], BF16, tag="ident")
            make_identity(nc, ident)
            eps_t = consts.tile([P, 1], F32, tag="eps")
            nc.vector.memset(eps_t, EPS)
            if ln_affine:
                gq_b = consts.tile([P, D], F32, tag="gq_b")
                nc.gpsimd.dma_start(out=gq_b,
                                    in_=gq_d[None, :].to_broadcast((P, D)))
                btq_b = consts.tile([P, D], F32, tag="btq_b")
                nc.gpsimd.dma_start(out=btq_b,
                                    in_=btq_d[None, :].to_broadcast((P, D)))
                gkv_b = consts.tile([P, D], F32, tag="gkv_b")
                nc.gpsimd.dma_start(out=gkv_b,
                                    in_=gkv_d[None, :].to_broadcast((P, D)))
                btkv_b = consts.tile([P, D], F32, tag="btkv_b")
                nc.gpsimd.dma_start(out=btkv_b,
                                    in_=btkv_d[None, :].to_broadcast((P, D)))
            else:
                gq_b = btq_b = gkv_b = btkv_b = None
            bv_b = consts.tile([P, DG], F32, tag="bv_b")
            nc.gpsimd.dma_start(out=bv_b, in_=bv_d[None, :].to_broadcast((P, DG)))
            bq_c = consts.tile([P, NM], F32, tag="bq_c")
            nc.sync.dma_start(out=bq_c, in_=bq_d.rearrange("(m p) -> p m", p=P))
            bk_c = consts.tile([P, NM], F32, tag="bk_c")
            nc.sync.dma_start(out=bk_c, in_=bk_d.rearrange("(m p) -> p m", p=P))

            # ---------------- weights (fp32 -> bf16) ----------------
            wq_bf = wpool.tile([P, KS, DG], BF16, tag="wq_bf")
            wk_bf = wpool.tile([P, KS, DG], BF16, tag="wk_bf")
            wv_bf = wpool.tile([P, KS, DG], BF16, tag="wv_bf")
            for wi, (w_d, w_bf) in enumerate(
                    ((wq, wq_bf), (wk, wk_bf), (wv, wv_bf))):
                w_r = w_d.rearrange("(a s p) n -> p a s n", p=P, s=4)
                for a in range(2):
                    st = wstage.tile([P, 4, DG], F32, tag="wst")
                    (nc.sync if (wi + a) % 2 == 0 else nc.scalar).dma_start(
                        out=st, in_=w_r[:, a])
                    nc.vector.tensor_copy(
                        out=w_bf[:, a * 4:(a + 1) * 4, :], in_=st)
            wo_r = wo.rearrange("(m p) n -> p m n", p=P)
            st = wstage.tile([P, 4, DG], F32, tag="wst")
            st2 = wstage.tile([P, 4, DG], F32, tag="wst")
            nc.sync.dma_start(out=st, in_=wo_r[:, :, 0:DG])
            nc.scalar.dma_start(out=st2, in_=wo_r[:, :, DG:D])
            nc.vector.tensor_copy(out=wo_bf[:, :, 0:DG], in_=st)
            nc.vector.tensor_copy(out=wo_bf[:, :, DG:D], in_=st2)

            # ---------------- LN + transpose + projections ----------------
            def ln_tile(xb, tl, g_b, b_b):
                """LayerNorm token tile xb[:, tl, :] -> [128, 1024] bf16."""
                x = xb[:, tl, :]
                st = stats.tile([P, 2, 6], F32, tag="bnst")
                nc.vector.bn_stats(out=st[:, 0, :], in_=x[:, 0:512])
                nc.vector.bn_stats(out=st[:, 1, :], in_=x[:, 512:1024])
                mv = stats.tile([P, 2], F32, tag="mv")
                nc.vector.bn_aggr(out=mv, in_=st)
                lnv = stats.tile([P, 1], F32, tag="lnv")
                nc.scalar.activation(out=lnv, in_=mv[:, 1:2],
                                     func=mybir.ActivationFunctionType.Ln,
                                     bias=eps_t)
                rstd = stats.tile([P, 1], F32, tag="rstd")
                nc.scalar.activation(out=rstd, in_=lnv,
                                     func=mybir.ActivationFunctionType.Exp,
                                     scale=-0.5)
                lnt = lnpool.tile([P, D], BF16, tag="ln")
                if not ln_affine:
                    nc.vector.tensor_scalar(out=lnt, in0=x, scalar1=mv[:, 0:1],
                                            scalar2=rstd,
                                            op0=mybir.AluOpType.subtract,
                                            op1=mybir.AluOpType.mult)
                    return lnt
                xc = lntmp.tile([P, D], F32, tag="xc")
                nc.vector.tensor_scalar(out=xc, in0=x, scalar1=mv[:, 0:1],
                                        scalar2=rstd,
                                        op0=mybir.AluOpType.subtract,
                                        op1=mybir.AluOpType.mult)
                xg = lntmp.tile([P, D], F32, tag="xg")
                nc.vector.tensor_tensor(out=xg, in0=xc, in1=g_b,
                                        op=mybir.AluOpType.mult)
                nc.vector.tensor_tensor(out=lnt, in0=xg, in1=b_b,
                                        op=mybir.AluOpType.add)
                return lnt

            def transpose_chunk(ln_tiles):
                """4 LN tiles ([128 tok, 1024 feat]) -> lnT [128 feat, 8, 512 tok]."""
                lnT = lntpool.tile([P, KS, 512], BF16, tag="lnT")
                for s in range(KS):
                    pt = ps_tr.tile([P, 512], BF16, tag="tr")
                    for tl in range(4):
                        nc.tensor.transpose(pt[:, tl * P:(tl + 1) * P],
                                            ln_tiles[tl][:, s * P:(s + 1) * P],
                                            ident)
                    nc.vector.tensor_copy(out=lnT[:, s, :], in_=pt)
                return lnT

            c_r = c_in.rearrange("(n i p) d -> n p i d", p=P, i=4)
            q_r = q_in.rearrange("(n i p) d -> n p i d", p=P, i=4)
            # query: Q^T
            for c in range(NCH):
                xb = xpool.tile([P, 4, D], F32, tag="xb")
                (nc.sync if c % 2 == 0 else nc.scalar).dma_start(
                    out=xb, in_=q_r[c])
                ln_tiles = [ln_tile(xb, tl, gq_b, btq_b)
                            for tl in range(4)]
                lnT = transpose_chunk(ln_tiles)
                for m in range(NM):
                    pp = ps_proj.tile([P, 512], F32, tag="pp")
                    for s in range(KS):
                        nc.tensor.matmul(pp, lhsT=wq_bf[:, s, m * P:(m + 1) * P],
                                         rhs=lnT[:, s, :],
                                         start=(s == 0), stop=(s == KS - 1))
                    nc.vector.tensor_scalar_add(
                        out=qt[c][:, m, :], in0=pp,
                        scalar1=bq_c[:, m:m + 1])


            # context: K^T, V
            for c in range(NCH):
                xb = xpool.tile([P, 4, D], F32, tag="xb")
                (nc.sync if c % 2 == 0 else nc.scalar).dma_start(
                    out=xb, in_=c_r[c])
                ln_tiles = [ln_tile(xb, tl, gkv_b, btkv_b)
                            for tl in range(4)]
                lnT = transpose_chunk(ln_tiles)
                for m in range(NM):
                    pp = ps_proj.tile([P, 512], F32, tag="pp")
                    for s in range(KS):
                        nc.tensor.matmul(pp, lhsT=wk_bf[:, s, m * P:(m + 1) * P],
                                         rhs=lnT[:, s, :],
                                         start=(s == 0), stop=(s == KS - 1))
                    nc.vector.tensor_scalar_add(
                        out=kt[c][:, m, :], in0=pp,
                        scalar1=bk_c[:, m:m + 1])
                for tl in range(4):
                    t = 4 * c + tl
                    pp = ps_proj.tile([P, 512], F32, tag="pp")
                    for s in range(KS):
                        nc.tensor.matmul(pp, lhsT=lnT[:, s, tl * P:(tl + 1) * P],
                                         rhs=wv_bf[:, s, :],
                                         start=(s == 0), stop=(s == KS - 1))
                    nc.vector.tensor_tensor(
                        out=vs[:, t, :, 0:HD],
                        in0=pp.rearrange("p (h d) -> p h d", h=NH),
                        in1=bv_b.rearrange("p (h d) -> p h d", h=NH),
                        op=mybir.AluOpType.add)

def _attn_phase(nc, tc, qt, kt, vs, os_t, wo_bf, ps_proj, y_out):
    with (
        tc.tile_pool(name="exp", bufs=3) as exppool,
        tc.tile_pool(name="smalls", bufs=2) as smalls,
        tc.tile_pool(name="yout", bufs=3) as ypool,
        tc.tile_pool(name="ps_s", bufs=2, space="PSUM") as ps_s,
        tc.tile_pool(name="ps_av", bufs=2, space="PSUM") as ps_av,
    ):
        uq = _UNIQ[0]

        def emit_av_chunk(prev, kg):
            c0, j0, exp_pair, avs = prev
            for hl in range(2):
                for k2 in range(2):
                    ki = kg * 2 + k2
                    nc.tensor.matmul(avs[hl], lhsT=vs[:, ki, 2 * j0 + hl, :],
                                     rhs=exp_pair[hl][:, ki, :],
                                     start=(ki == 0), stop=(ki == NT - 1),
                                     skip_group_check=True)

        def emit_normalize(prev):
            c0, j0, exp_pair, avs = prev
            for hl in range(2):
                av = avs[hl]
                zrow = smalls.tile([1, 512], F32, tag="zrow",
                                   name=f"zrow{c0}_{j0}_{hl}_{uq}")
                nc.vector.reciprocal(out=zrow, in_=av[HD:HD + 1, :])
                rinv = smalls.tile([HD, 512], F32, tag="rinv",
                                   name=f"rinv{c0}_{j0}_{hl}_{uq}")
                nc.gpsimd.partition_broadcast(rinv, zrow)
                nc.vector.tensor_tensor(
                    out=os_t[c0][hl * HD:(hl + 1) * HD, j0, :],
                    in0=av[0:HD, :], in1=rinv,
                    op=mybir.AluOpType.mult)

        yt_cur = [None]

        def emit_wo_group(c0, g):
            tl, dc = g // 2, g % 2
            t = 4 * c0 + tl
            pp = ps_proj.tile([P, 512], F32, tag="pp",
                              name=f"wopp{c0}_{g}_{uq}")
            for m in range(NM):
                nc.tensor.matmul(
                    pp, lhsT=os_t[c0][:, m, tl * P:(tl + 1) * P],
                    rhs=wo_bf[:, m, dc * 512:(dc + 1) * 512],
                    start=(m == 0), stop=(m == NM - 1),
                    skip_group_check=True)
            if dc == 0:
                yt_cur[0] = ypool.tile([P, 2, 512], F32, tag="y",
                                       name=f"yt{c0}_{g}_{uq}")
            nc.vector.tensor_copy(out=yt_cur[0][:, dc, :], in_=pp)
            if dc == 1:
                nc.sync.dma_start(
                    out=y_out[t * P:(t + 1) * P, :], in_=yt_cur[0])

        prev = None
        wo_pending = []   # (c, next_group_idx)
        for c in range(NCH):
            for j in range(NM):
                exp_pair = [exppool.tile([P, NT, 512], BF16, tag=f"exp{hl}",
                                         name=f"exp{hl}_{c}_{j}_{uq}")
                            for hl in range(2)]
                for kg in range(8):
                    ps_pair = [ps_s.tile([P, 2, 512], F32, tag="psS",
                                         name=f"psS{hl}_{c}_{j}_{kg}_{uq}")
                               for hl in range(2)]
                    for k2 in range(2):
                        ki = kg * 2 + k2
                        for hl in range(2):
                            rows = slice(hl * HD, (hl + 1) * HD)
                            nc.tensor.matmul(
                                ps_pair[hl][:, k2, :],
                                lhsT=kt[ki // 4][rows, j,
                                          (ki % 4) * P:(ki % 4 + 1) * P],
                                rhs=qt[c][rows, j, :],
                                start=True, stop=True,
                                skip_group_check=True)
                    for hl in range(2):
                        if _EXP_ON_DVE[0]:
                            nc.vector.tensor_copy(
                                out=exp_pair[hl][:, kg * 2:kg * 2 + 2, :],
                                in_=ps_pair[hl][:, :, :])
                        else:
                            nc.scalar.activation(
                                out=exp_pair[hl][:, kg * 2:kg * 2 + 2, :],
                                in_=ps_pair[hl][:, :, :],
                                func=mybir.ActivationFunctionType.Exp,
                                scale=SCALE)
                    if prev is not None:
                        emit_av_chunk(prev, kg)
                    if wo_pending and kg in (3, 7):
                        c0, g = wo_pending[0]
                        emit_wo_group(c0, g)
                        if g + 1 >= 8:
                            wo_pending.pop(0)
                        else:
                            wo_pending[0] = (c0, g + 1)
                if prev is not None:
                    emit_normalize(prev)
                    if prev[1] == NM - 1:      # finished batch-chunk prev[0]
                        wo_pending.append((prev[0], 0))
                avs = [ps_av.tile([HD + 1, 512], F32, tag="av",
                                  name=f"av{c}_{j}_{hl}_{uq}")
                       for hl in range(2)]
                prev = (c, j, exp_pair, avs)
        # drain: AV + normalize of the last (c,j), then remaining Wo groups
        for kg in range(8):
            emit_av_chunk(prev, kg)
        emit_normalize(prev)
        wo_pending.append((prev[0], 0))
        for c0, g0 in list(wo_pending):
            for g in range(g0, 8):
                emit_wo_group(c0, g)


_CACHE = {}


def _get_exec(ln_affine=True, repeat=1, hw_loop=0):
    """Build the Bass program once and wrap it in a reusable jitted executor."""
    key = ("exec", ln_affine, repeat, hw_loop)
    if key in _CACHE:
        return _CACHE[key]

    import jax
    from jax.sharding import Mesh, PartitionSpec
    from jax.experimental.shard_map import shard_map
    from concourse import bass2jax

    nc = _build_program(ln_affine=ln_affine, repeat=repeat, hw_loop=hw_loop)
    bass2jax.install_neuronx_cc_hook()

    partition_name = (nc.partition_id_tensor.name
                      if nc.partition_id_tensor else None)
    in_names, out_names, out_avals, zero_shapes = [], [], [], []
    for alloc in nc.m.functions[0].allocations:
        if not isinstance(alloc, mybir.MemoryLocationSet):
            continue
        name = alloc.memorylocations[0].name
        if alloc.kind == "ExternalInput":
            if name != partition_name:
                in_names.append(name)
        elif alloc.kind == "ExternalOutput":
            shape = tuple(alloc.tensor_shape)
            dtype = mybir.dt.np(alloc.dtype)
            out_names.append(name)
            out_avals.append(jax.core.ShapedArray(shape, dtype))
            zero_shapes.append((shape, dtype))
    n_params = len(in_names)
    n_outs = len(out_avals)
    all_names = list(in_names) + list(out_names)
    if partition_name is not None:
        all_names.append(partition_name)
    donate = tuple(range(n_params, n_params + n_outs))

    def _body(*args):
        operands = list(args)
        if partition_name is not None:
            operands.append(bass2jax.partition_id_tensor())
        outs = bass2jax._bass_exec_p.bind(
            *operands,
            out_avals=tuple(out_avals),
            in_names=tuple(all_names),
            out_names=tuple(out_names),
            lowering_input_output_aliases=(),
            sim_require_finite=True,
            sim_require_nnan=True,
            nc=nc,
        )
        return tuple(outs)

    n_cores = 8
    devices = jax.devices()[:n_cores]
    mesh = Mesh(np.asarray(devices), ("core",))
    in_specs = (PartitionSpec("core"),) * (n_params + n_outs)
    out_specs = (PartitionSpec("core"),) * n_outs
    sharded = jax.jit(
        shard_map(_body, mesh=mesh, in_specs=in_specs, out_specs=out_specs,
                  check_rep=False),
        donate_argnums=donate, keep_unused=True)

    def execute(in_maps):
        per_core = [[np.ascontiguousarray(np.asarray(m[name], np.float32))
                     for name in in_names] for m in in_maps]
        concat_in = [np.concatenate([per_core[cc][i] for cc in range(n_cores)],
                                    axis=0) for i in range(n_params)]
        concat_zeros = [np.zeros((n_cores * s[0], *s[1:]), d)
                        for (s, d) in zero_shapes]
        out_arrs = sharded(*concat_in, *concat_zeros)
        return [
            {name: np.asarray(out_arrs[i]).reshape(n_cores, *out_avals[i].shape)[cc]
             for i, name in enumerate(out_names)}
            for cc in range(n_cores)
        ]

    _CACHE[key] = execute
    _CACHE[("parts", ln_affine, repeat, hw_loop)] = {
        "sharded": sharded, "in_names": in_names, "n_params": n_params,
        "out_names": out_names, "out_avals": out_avals,
        "zero_shapes": zero_shapes, "mesh": mesh, "n_cores": n_cores,
        "body": _body, "in_specs": in_specs, "out_specs": out_specs,
        "donate": donate,
    }
    return execute


def _time_exec(in_maps, iters=5, ln_affine=True, repeat=1, hw_loop=0):
    """Time the sharded executable with device-resident inputs (seconds)."""
    import time
    import jax
    from jax.sharding import NamedSharding, PartitionSpec

    _get_exec(ln_affine=ln_affine, repeat=repeat, hw_loop=hw_loop)
    parts = _CACHE[("parts", ln_affine, repeat, hw_loop)]
    sharded = parts["sharded"]
    n_cores = parts["n_cores"]
    sh = NamedSharding(parts["mesh"], PartitionSpec("core"))
    per_core = [[np.ascontiguousarray(np.asarray(m[name], np.float32))
                 for name in parts["in_names"]] for m in in_maps]
    concat_in = [np.concatenate([per_core[cc][i] for cc in range(n_cores)],
                                axis=0) for i in range(parts["n_params"])]
    in_dev = [jax.device_put(a, sh) for a in concat_in]
    jax.block_until_ready(in_dev)
    times = []
    for _ in range(iters):
        z_dev = [jax.device_put(
                     np.zeros((n_cores * s[0], *s[1:]), d), sh)
                 for (s, d) in parts["zero_shapes"]]
        jax.block_until_ready(z_dev)
        t0 = time.perf_counter()
        out = sharded(*in_dev, *z_dev)
        jax.block_until_ready(out)
        times.append(time.perf_counter() - t0)
        del out
    return times


def _ln_is_identity(inputs):
    return all(
        np.all(np.asarray(inputs[k], np.float32) == v)
        for k, v in (("gq", 1.0), ("betq", 0.0), ("gkv", 1.0), ("betkv", 0.0))
    )


def _make_in_maps(inputs):
    q = np.asarray(inputs["query"], np.float32)
    c = np.asarray(inputs["context"], np.float32)
    Wq = np.asarray(inputs["Wq"], np.float32)
    Wk = np.asarray(inputs["Wk"], np.float32)
    Wv = np.asarray(inputs["Wv"], np.float32)
    Wo = np.asarray(inputs["Wo"], np.float32)
    bq = np.asarray(inputs["bq"], np.float32)
    bk = np.asarray(inputs["bk"], np.float32)
    bv = np.asarray(inputs["bv"], np.float32)
    gq = np.asarray(inputs["gq"], np.float32)
    btq = np.asarray(inputs["betq"], np.float32)
    gkv = np.asarray(inputs["gkv"], np.float32)
    btkv = np.asarray(inputs["betkv"], np.float32)
    in_maps = []
    for core in range(8):
        b, hg = core // 2, core % 2
        sl = slice(hg * DG, (hg + 1) * DG)
        in_maps.append({
            "q_in": q[b], "c_in": c[b],
            "wq": Wq[:, sl], "wk": Wk[:, sl], "wv": Wv[:, sl],
            "wo": Wo[sl, :],
            "bq": bq[sl], "bk": bk[sl], "bv": bv[sl],
            "gq": gq, "btq": btq, "gkv": gkv, "btkv": btkv,
        })
    return in_maps


def kernel(**inputs):
    execute = _get_exec(ln_affine=not _ln_is_identity(inputs))
    in_maps = _make_in_maps(inputs)
    results = execute(in_maps)
    bo = np.asarray(inputs["bo"], np.float32)
    B = 4
    out = np.empty((B, N_TOK, D), np.float32)
    for b in range(B):
        out[b] = results[2 * b]["y_out"] + results[2 * b + 1]["y_out"] + bo
    return out


def _dma_only(nc, tc, q_in, c_in, wq, wk, wv, wo, y_out):
    mode = _DMA_MODE[0]
    with tc.tile_pool(name="dml", bufs=4) as pool:
        acc = pool.tile([P, D], F32, tag="acc")
        if mode == 2:
            for src in (q_in, c_in):
                r4 = src.rearrange("(n i p) d -> n p i d", p=P, i=4)
                for n in range(NT // 4):
                    xb = pool.tile([P, 4, D], F32, tag="dxb")
                    eng = nc.sync if n % 2 == 0 else nc.scalar
                    eng.dma_start(out=xb, in_=r4[n])
                    nc.vector.tensor_copy(out=acc, in_=xb[:, 0, :])
        else:
            for src in (q_in, c_in):
                for t in range(NT):
                    x = pool.tile([P, D], F32, tag="dx")
                    eng = nc.sync if (mode == 0 or t % 2 == 0) else nc.scalar
                    eng.dma_start(out=x, in_=src[t * P:(t + 1) * P, :])
                    nc.vector.tensor_copy(out=acc, in_=x)
        for w_d in (wq, wk, wv):
            w_r = w_d.rearrange("(s p) n -> p s n", p=P)
            for s_ in range(KS):
                x = pool.tile([P, D], F32, tag="dx")
                nc.sync.dma_start(out=x[:, :DG], in_=w_r[:, s_, :])
                nc.vector.tensor_copy(out=acc, in_=x)
        wo_r = wo.rearrange("(m p) n -> p m n", p=P)
        for m in range(NM):
            x = pool.tile([P, D], F32, tag="dx")
            nc.sync.dma_start(out=x, in_=wo_r[:, m, :])
            nc.vector.tensor_copy(out=acc, in_=x)
        nc.sync.dma_start(out=y_out[0:P, 0:D], in_=acc)
